# revision 1
# baseline (speedup 1.0000x reference)
"""Trainium2 Bass kernel for nn_KLDLoss_18769007083961.

Math reformulation (validated vs reference, rel err ~1e-6):
  For each image b, prototype a with class c(a), define over pixels p:
    s_a[p]  = d_a[p] + (label[p] != c(a)) * (-1e4)      # masked-biased distance
    em_a[p] = exp(s_a[p])                               # exactly 0 off-class (f32 underflow)
    Z_a     = sum_p em_a[p]
    G[a,j]  = sum_p em_a[p] * s_j[p]   (j in same group => same class mask)
    A[a,j]  = G[a,j] / Z_a
  Symmetric KL for a same-group pair (i,j) (log-partition terms cancel):
    kld = 0.5 * (A[j,j] - A[j,i] + A[i,i] - A[i,j])
  loss = mean over valid pairs (class count >= 2) of exp(-kld).

Device kernel (one image per NeuronCore, 8 cores):
  Layout: pixel p = 512*q + 128*w + i  (q = SBUF partition, w = window, i = inner).
  Per window: DMA dist -> s_tile[128, 81*128] (class-major proto order, slot 80 = 1.0),
  DVE builds the -1e4 class bias, ACT computes em = exp(s), then 128 matmuls
  (lhsT = s-slice [128,81], rhs = em-slice [128,80]) accumulate out[m,n] =
  sum_p s_m * em_n into PSUM [81,80]: rows 0..79 = G[n,m]... i.e. out[j,a] = G[a,j],
  row 80 = Z.  Host does the tiny 120-pair combination.
"""

import sys
from contextlib import ExitStack

import numpy as np

sys.path.insert(0, "/opt/trn_rl_repo")

import concourse.bass as bass
import concourse.tile as tile
from concourse import mybir
from concourse.bass_utils import run_bass_kernel_spmd
from concourse.tile import add_dep_helper

B = 8
C = 10
NPROT = 80
P = 65536
Q = 128          # partitions = coarse pixel blocks of 512
W = 4            # windows per image
FI = 128         # inner pixels per window per partition
F32 = mybir.dt.float32

_NC_CACHE = {}


def build_nc():
    nc = bass.Bass()
    # 81 rows: 80 prototypes + a constant-1.0 row that lands in the ones slot
    d_in = nc.dram_tensor("dist", [NPROT + 1, P], F32, kind="ExternalInput")
    # labels [q, 512] packed with the 10 class constants -> cols 512..521
    lab_in = nc.dram_tensor("labcls", [Q, 512 + C], F32, kind="ExternalInput")
    g_out = nc.dram_tensor("g", [81, 80], F32, kind="ExternalOutput")

    with ExitStack() as ctx:
        tc = ctx.enter_context(tile.TileContext(nc))
        singles = ctx.enter_context(tc.tile_pool(name="singles", bufs=1))
        spool = ctx.enter_context(tc.tile_pool(name="spool", bufs=2))
        empool = ctx.enter_context(tc.tile_pool(name="empool", bufs=2))
        mpool = ctx.enter_context(tc.tile_pool(name="mpool", bufs=2))
        psum = ctx.enter_context(tc.tile_pool(name="psum", bufs=1, space="PSUM"))

        labels_t = singles.tile([Q, 512 + C], F32)
        nc.sync.dma_start(out=labels_t, in_=lab_in[:, :])
        cls_t = labels_t[:, 512 : 512 + C]

        g_ps = psum.tile([81, 80], F32)

        # dist[n, p] with p = 512*q + 128*w + i ; natural proto order n = 40*s+4*c+m
        dview = d_in.rearrange("n (q w i) -> n q w i", q=Q, w=W, i=FI)

        first = True
        em_tiles = []
        # Windows 0/1 go to fresh buffers -> plain SP DMAs with no WAR waits.
        # Windows 2/3 recycle buffers; their DMAs are issued from the ACT
        # sequencer right after exp(w-1) (see bottom of the loop), where ACT's
        # clock has already observed the DVE/DMAHW ticks, leaving one PE wait.
        s_tiles = []
        for w in range(2):
            s_w = spool.tile([Q, 81 * FI], F32, tag="s", name=f"s_t{w}")
            nc.sync.dma_start(
                out=s_w.rearrange("p (n i) -> p n i", n=81),
                in_=dview[:, :, w, :].transpose([1, 0, 2]),
            )
            s_tiles.append(s_w)
        for w in range(W):
            s_t = s_tiles[w]

            # mne[p, c, i] = (labels != c) as 1.0/0.0
            mne = mpool.tile([Q, C * FI], F32, tag="mne")
            lab_w = labels_t[:, w * FI : (w + 1) * FI]
            nc.vector.tensor_tensor(
                mne.rearrange("p (c i) -> p c i", c=C),
                lab_w.unsqueeze(1).broadcast_to([Q, C, FI]),
                cls_t.unsqueeze(2).broadcast_to([Q, C, FI]),
                mybir.AluOpType.not_equal,
            )

            # Absorb the dist-DMA completion into DVE's clock with a 1-element
            # copy so the first STT below needs only the mne (DVE) wait.
            probe = mpool.tile([Q, 1], F32, tag="probe", bufs=4)
            nc.vector.tensor_copy(probe, s_t[:, 0:1])
            if w >= 2:
                # Buf recycling gives the first s_t writer WAR deps on both
                # ACT (exp read) and PE (lhsT read) of window w-2.  DVE
                # instructions have a single wait slot, so absorb each dep
                # with its own 1-element op against the old em tile: a read
                # observes ACT, a write observes PE's rhs read.
                em_old = em_tiles[w - 2]
                probe2 = mpool.tile([Q, 1], F32, tag="probe2", bufs=4)
                nc.vector.tensor_copy(probe2, em_old[:, 0:1])
                # disjoint bytes from probe2's read so no same-engine WAR wait
                nc.vector.memset(em_old[:, 1:2], 0.0)

            # s = (mne * -1e4) + d   (in place; walrus caps compute APs at 3 dims,
            # so one op per (scale, class): out [p, 4*FI], in0 [p, m(bcast), i])
            mne_v = mne.rearrange("p (c i) -> p c i", c=C)
            for sc in range(2):
                for c in range(C):
                    n0 = 40 * sc + 4 * c
                    s_dat = s_t[:, n0 * FI : (n0 + 4) * FI]
                    mne_b = mne_v[:, c].unsqueeze(1).broadcast_to([Q, 4, FI])
                    nc.vector.scalar_tensor_tensor(
                        s_dat,
                        mne_b,
                        -1.0e4,
                        s_dat,
                        mybir.AluOpType.mult,
                        mybir.AluOpType.add,
                    )

            # ACT-side absorbers (ACT structs also have one wait slot).  The
            # ones-slot byte is written ONLY by the DMA, so this copy carries
            # just the DMAHW wait.
            dead_act = mpool.tile([Q, 1], F32, tag="dead_act", bufs=4)
            i_abs1 = nc.scalar.copy(dead_act, s_t[:, 80 * FI : 80 * FI + 1])
            act_absorbers = [i_abs1]
            if w >= 2:
                # exp(w) overwrites em(w-2): absorb the WAW-vs-old-exp (ACT
                # sem) by reading an old-em byte, and the WAR-vs-PE-rhs-reads
                # by reading the PSUM accumulator (PE's only visible output).
                dead3 = mpool.tile([Q, 1], F32, tag="dead3", bufs=4)
                if w == 2:
                    src3 = em_tiles[w - 2][:, 2:3]
                else:
                    # reading the previous dead4 absorbs both the old-exp WAW
                    # tick and the PSUM reader-reader serialization tick
                    src3 = last_dead4[0:1, 0:1]
                act_absorbers.append(nc.scalar.copy(dead3[: src3.shape[0]], src3))
                dead4 = mpool.tile([1, 1], F32, tag="dead4", bufs=4)
                act_absorbers.append(nc.scalar.copy(dead4, g_ps[0:1, 0:1]))
                last_dead4 = dead4

            # em = exp(s) (slot 80 -> exp(1), unused by rhs)
            em_t = empool.tile([Q, 81 * FI], F32, tag="em")
            em_tiles.append(em_t)
            i_exp = nc.scalar.activation(em_t, s_t, mybir.ActivationFunctionType.Exp)
            for a in act_absorbers:
                add_dep_helper(i_exp.ins, a.ins, sync=False)

            if w + 1 >= 2 and w + 1 < W:
                s_next = spool.tile([Q, 81 * FI], F32, tag="s", name=f"s_t{w+1}")
                i_dma = nc.scalar.dma_start(
                    out=s_next.rearrange("p (n i) -> p n i", n=81),
                    in_=dview[:, :, w + 1, :].transpose([1, 0, 2]),
                )
                add_dep_helper(i_dma.ins, i_exp.ins, sync=False)
                s_tiles.append(s_next)

            # PE-side absorbers: LDW/MM structs also have a small wait budget,
            # so acquire the DMA then the ACT tick with 1x1 dummy matmuls; the
            # real matmuls then carry only the DVE wait.
            ones_col = s_t[:, 80 * FI : 80 * FI + 1]
            if w == 0:
                dummy_ps = psum.tile([1, 1], F32, tag="dummy", bufs=1)
                dummy_ps2 = psum.tile([1, 1], F32, tag="dummy2", bufs=1)
            i_pabs1 = nc.tensor.matmul(
                dummy_ps, ones_col, ones_col, start=(w == 0), stop=(w == W - 1),
                skip_group_check=True,
            )
            i_pabs2 = nc.tensor.matmul(
                dummy_ps2, ones_col, em_t[:, 0:1], start=(w == 0), stop=(w == W - 1),
                skip_group_check=True,
            )
            add_dep_helper(i_pabs2.ins, i_pabs1.ins, sync=False)

            s_mm = s_t.rearrange("p (n i) -> p n i", n=81)
            em_mm = em_t.rearrange("p (n i) -> p n i", n=81)
            for i in range(FI):
                i_mm = nc.tensor.matmul(
                    g_ps,
                    s_mm[:, :, i],
                    em_mm[:, :80, i],
                    start=first,
                    stop=(w == W - 1 and i == FI - 1),
                )
                if i == 0:
                    add_dep_helper(i_mm.ins, i_pabs2.ins, sync=False)
                first = False

        # DVE absorber for the ACT PSUM-read serialization, so the final
        # PSUM->SBUF copy carries only the PE wait.
        deadf = mpool.tile([1, 1], F32, tag="deadf", bufs=1)
        i_fabs = nc.vector.tensor_copy(deadf, last_dead4)
        g_sb = singles.tile([81, 80], F32)
        i_gcopy = nc.vector.tensor_copy(g_sb, g_ps)
        add_dep_helper(i_gcopy.ins, i_fabs.ins, sync=False)
        nc.sync.dma_start(out=g_out[:, :], in_=g_sb)

    # The kernel-tail drain aggregates every outstanding semaphore into one
    # instruction; the CTRL struct cannot hold that many waits.  Split it
    # into a chain of single-wait drains.
    import copy as _copy

    for fn in nc.m.functions:
        for blk in fn.blocks:
            insts = blk.instructions
            for idx, ins in enumerate(list(insts)):
                si = ins.sync_info
                if type(ins).__name__ == "InstDrain" and si and len(si.on_wait) > 1:
                    waits = list(si.on_wait)
                    si.on_wait = waits[-1:]
                    pos = insts.index(ins)
                    for k, wt in enumerate(waits[:-1]):
                        d2 = _copy.deepcopy(ins)
                        d2.name = f"{ins.name}-split{k}"
                        d2.sync_info = type(si)(on_wait=[wt], on_update=[])
                        insts.insert(pos + k, d2)
                    break

    return nc


def _get_nc():
    if "nc" not in _NC_CACHE:
        _NC_CACHE["nc"] = build_nc()
    return _NC_CACHE["nc"]


def run_device(dist8, labf8, trace=False):
    """dist8: [8, 81, P] f32 permuted + ones row; labf8: [8, P] f32 labels-1."""
    nc = _get_nc()
    cls = np.broadcast_to(np.arange(C, dtype=np.float32)[None, :], (Q, C))
    in_maps = []
    for b in range(B):
        labcls = np.concatenate([labf8[b].reshape(Q, 512), cls], axis=1)
        in_maps.append(
            {"dist": dist8[b], "labcls": np.ascontiguousarray(labcls)}
        )
    return run_bass_kernel_spmd(nc, in_maps, list(range(B)), trace=trace)


def kernel(
    prototype_distances,
    target_labels,
    proto_class,
    pair_i,
    pair_j,
    pair_cls,
    _trace=False,
    _results_out=None,
):
    dist = np.asarray(prototype_distances, dtype=np.float32).reshape(B, NPROT, P)
    labels = np.asarray(target_labels).reshape(B, P).astype(np.int64)
    proto_class = np.asarray(proto_class, dtype=np.int64)
    pair_i = np.asarray(pair_i, dtype=np.int64)
    pair_j = np.asarray(pair_j, dtype=np.int64)
    pair_cls = np.asarray(pair_cls, dtype=np.int64)

    # Permute prototypes so the device's assumed class layout (n%40)//4 holds.
    target_cls = (np.arange(NPROT) % 40) // 4
    perm = np.empty(NPROT, dtype=np.int64)
    for c in range(C):
        protos = np.nonzero(proto_class == c)[0]
        slots = np.nonzero(target_cls == c)[0]
        assert len(protos) == len(slots) == 8, "expect 8 prototypes per class"
        perm[slots] = protos
    inv = np.empty(NPROT, dtype=np.int64)
    inv[perm] = np.arange(NPROT)

    dist_p = np.empty((B, NPROT + 1, P), dtype=np.float32)
    dist_p[:, :NPROT, :] = dist[:, perm, :]
    dist_p[:, NPROT, :] = 1.0
    labf = np.ascontiguousarray((labels - 1).astype(np.float32))

    br = run_device(dist_p, labf, trace=_trace)
    if _results_out is not None:
        _results_out.append(br)

    total_vals = np.float64(0.0)
    total_valid = 0
    for b in range(B):
        out = br.results[b]["g"]  # [81, 80]; out[j, a] = G[a, j], out[80, a] = Z_a
        Z = out[80].astype(np.float64)
        Gt = out[:80].astype(np.float64)  # Gt[j, a] = sum_p em_a * s_j
        with np.errstate(divide="ignore", invalid="ignore"):
            A = np.where(Z[None, :] != 0.0, Gt / Z[None, :], 0.0)  # A[j, a] = E_a[d_j]
        lb = labels[b] - 1
        cnt = np.bincount(lb[lb >= 0], minlength=C)
        ii = inv[pair_i]
        jj = inv[pair_j]
        # A[x, a] = expectation of d_x under softmax of proto a
        kld = 0.5 * (A[jj, jj] - A[jj, ii] + A[ii, ii] - A[ii, jj])
        valid = cnt[pair_cls] >= 2
        total_vals += np.exp(-kld[valid]).sum()
        total_valid += int(valid.sum())

    if total_valid > 0:
        res = np.float32(total_vals / max(total_valid, 1))
    else:
        res = np.float32(0.0)
    return res


if __name__ == "__main__":
    rng = np.random.default_rng(0)
    d = rng.standard_normal((B, NPROT, 256, 256), dtype=np.float32)
    l = rng.integers(0, 11, (B, 256, 256))
    pc = (np.arange(NPROT) % 40) // 4
    pairs = []
    for s in range(2):
        for c in range(C):
            base = s * 40 + c * 4
            for a in range(4):
                for b2 in range(a + 1, 4):
                    pairs.append((base + a, base + b2, c))
    pairs = np.asarray(pairs, np.int32)
    print(kernel(d, l, pc, pairs[:, 0], pairs[:, 1], pairs[:, 2]))



# revision 15
# speedup vs baseline: 1.6489x; 1.6489x over previous
"""Trainium2 Bass kernel for nn_KLDLoss_18769007083961.

Math reformulation (validated vs reference, rel err ~3e-5 with bf16):
  For each image b, prototype a with class c(a), define over pixels p:
    s_a[p]  = d_a[p] + (label[p] != c(a)) * (-1e4)      # masked-biased distance
    em_a[p] = exp(s_a[p])                               # exactly 0 off-class (underflow)
    Z_a     = sum_p em_a[p]
    G[a,j]  = sum_p em_a[p] * s_j[p]   (j in same group => same class mask)
    A[a,j]  = G[a,j] / Z_a
  Symmetric KL for a same-group pair (i,j) (log-partition terms cancel):
    kld = 0.5 * (A[j,j] - A[j,i] + A[i,i] - A[i,j])
  loss = mean over valid pairs (class count >= 2) of exp(-kld).

Only same-class G entries are consumed, and em is exactly zero off-class,
so the bf16 rounding of the -1e4 bias in s never reaches the result: the
biased s tile can be written once in bf16 and feed both the exp (ACT) and
the matmul lhsT (PE).  bf16 matmul runs at 1 cycle/row vs fp32's 4.

Device kernel (one image per NeuronCore, 8 cores):
  Layout: pixel p = 512*q + 128*w + i  (q = SBUF partition, w = window, i = inner).
  Protos are permuted host-side to class-major order (slot = 8c + 4s + m) so the
  class bias applies to 8 contiguous protos per DVE op.
  Per window: DMA dist -> s_t[128, 80*128] f32; DVE builds (label != c) and
  writes s16[128, 81*128] bf16 = d + mne*-1e4 (slot 80 memset to 1.0);
  ACT computes em = exp(s16) bf16; 128 matmuls (lhsT = s16-slice [128,81],
  rhs = em-slice [128,80]) accumulate out[j,a] = G[a,j], row 80 = Z, in
  PSUM [81,80] f32.  The last window runs in 4 column chunks to shorten the
  pipeline tail after the final DMA.  Host does the tiny 120-pair combine.
"""

import sys
from contextlib import ExitStack

import numpy as np

sys.path.insert(0, "/opt/trn_rl_repo")

import concourse.bass as bass
import concourse.tile as tile
from concourse import mybir
from concourse.bass_utils import run_bass_kernel_spmd
from concourse.tile import add_dep_helper

B = 8
C = 10
NPROT = 80
NSLOT = NPROT + 1  # 80 protos + ones column for Z
P = 65536
Q = 128          # partitions = coarse pixel blocks of 512
W = 4            # windows per image
FI = 128         # inner pixels per window per partition
F32 = mybir.dt.float32
BF16 = mybir.dt.bfloat16

_NC_CACHE = {}


def build_nc():
    nc = bass.Bass()
    d_in = nc.dram_tensor("dist", [NPROT, P], F32, kind="ExternalInput")
    # labels [q, 512] packed with the 10 class constants -> cols 512..521
    lab_in = nc.dram_tensor("labcls", [Q, 512 + C], F32, kind="ExternalInput")
    g_out = nc.dram_tensor("g", [NSLOT, NPROT], F32, kind="ExternalOutput")

    with ExitStack() as ctx:
        tc = ctx.enter_context(tile.TileContext(nc))
        singles = ctx.enter_context(tc.tile_pool(name="singles", bufs=1))
        spool = ctx.enter_context(tc.tile_pool(name="spool", bufs=2))
        s16pool = ctx.enter_context(tc.tile_pool(name="s16pool", bufs=2))
        empool = ctx.enter_context(tc.tile_pool(name="empool", bufs=2))
        mpool = ctx.enter_context(tc.tile_pool(name="mpool", bufs=2))
        psum = ctx.enter_context(tc.tile_pool(name="psum", bufs=1, space="PSUM"))

        labels_t = singles.tile([Q, 512 + C], F32)
        nc.sync.dma_start(out=labels_t, in_=lab_in[:, :])
        cls_t = labels_t[:, 512 : 512 + C]

        g_ps = psum.tile([NSLOT, NPROT], F32)

        # dist[n, p] with p = 512*q + 128*w + i ; class-major proto order
        dview = d_in.rearrange("n (q w i) -> n q w i", q=Q, w=W, i=FI)

        s_tiles = []
        for w in range(2):
            s_w = spool.tile([Q, NPROT * FI], F32, tag="s", name=f"s_t{w}")
            nc.sync.dma_start(
                out=s_w.rearrange("p (n i) -> p n i", n=NPROT),
                in_=dview[:, :, w, :].transpose([1, 0, 2]),
            )
            s_tiles.append(s_w)

        # constant source for the ACT-side absorber writes below
        zconst = singles.tile([Q, 1], BF16)
        nc.vector.memset(zconst, 0.0)

        # Engines have a single sync-wait slot per instruction.  Every
        # cross-engine dependency is therefore carried by a dedicated
        # 1-element absorber op, pinned ahead of its consumer with no-sync
        # dep edges so the scheduler keeps the elision-enabling order.
        first = True
        em_tiles = []
        dead4_tiles = []
        dead4_insts = []
        for w in range(W):
            s_t = s_tiles[w]
            s_v = s_t.rearrange("p (n i) -> p n i", n=NPROT)

            # mne[p, c, i] = (labels != c) as 1.0/0.0, bf16
            mne = mpool.tile([Q, C * FI], BF16, tag="mne")
            mne_v = mne.rearrange("p (c i) -> p c i", c=C)
            lab_w = labels_t[:, w * FI : (w + 1) * FI]
            nc.vector.tensor_tensor(
                mne_v,
                lab_w.unsqueeze(1).broadcast_to([Q, C, FI]),
                cls_t.unsqueeze(2).broadcast_to([Q, C, FI]),
                mybir.AluOpType.not_equal,
            )

            # DVE absorber chain: (1) dist-DMA completion for this window
            probe = mpool.tile([Q, 1], F32, tag="probe", bufs=4)
            i_probe = nc.vector.tensor_copy(probe, s_t[:, 0:1])
            dve_prev = i_probe
            if w >= 2:
                # (2) ACT finished exp(w-2), which read the s16 buffer the
                # STTs below recycle
                probe2 = mpool.tile([Q, 1], BF16, tag="probe2", bufs=4)
                i_probe2 = nc.vector.tensor_copy(probe2, em_tiles[w - 2][:, 0:1])
                add_dep_helper(i_probe2.ins, dve_prev.ins, sync=False)
                dve_prev = i_probe2

            s16 = s16pool.tile([Q, NSLOT * FI], BF16, tag="s16")
            s16_v = s16.rearrange("p (n i) -> p n i", n=NSLOT)
            em = empool.tile([Q, NPROT * FI], BF16, tag="em")
            em_v = em.rearrange("p (n i) -> p n i", n=NPROT)
            em_tiles.append(em)

            # ones column (slot 80) -> Z row of the gram.  For w >= 2 its
            # bytes were read by every LDW of window w-2, so this memset
            # carries exactly the "PE done with window w-2" wait that the
            # STTs would otherwise each need.
            i_memset = nc.vector.memset(s16_v[:, NPROT, :], 1.0)
            add_dep_helper(i_memset.ins, dve_prev.ins, sync=False)
            dve_prev = i_memset

            # ACT absorber: reading an old-em byte absorbs the same-engine
            # WAW tick (exp(w) overwrites exp(w-2)'s output); the PE tick was
            # absorbed by dead_act at the end of window w-2.
            act_abs = None
            if w >= 2:
                dead3 = mpool.tile([Q, 1], BF16, tag="dead3", bufs=2)
                act_abs = nc.scalar.copy(dead3, em_tiles[w - 2][:, 2:3])
                add_dep_helper(act_abs.ins, dead4_insts[w - 2].ins, sync=False)

            # last window in 4 chunks to shorten the tail after the final DMA
            nchunk = 4 if w == W - 1 else 1
            cw = FI // nchunk
            for k in range(nchunk):
                i0 = k * cw
                # s16 = (mne * -1e4) + d, bf16 out, 8 protos per class block
                for c in range(C):
                    n0 = 8 * c
                    mne_b = (
                        mne_v[:, c, i0 : i0 + cw]
                        .unsqueeze(1)
                        .broadcast_to([Q, 8, cw])
                    )
                    i_stt = nc.vector.scalar_tensor_tensor(
                        s16_v[:, n0 : n0 + 8, i0 : i0 + cw],
                        mne_b,
                        -1.0e4,
                        s_v[:, n0 : n0 + 8, i0 : i0 + cw],
                        mybir.AluOpType.mult,
                        mybir.AluOpType.add,
                    )
                    if k == 0 and c == 0:
                        add_dep_helper(i_stt.ins, dve_prev.ins, sync=False)

                act_prev = act_abs
                act_abs = None
                if w >= 2:
                    # ACT-side observer of the last STT of this chunk: exp
                    # below then sheds its DVE wait and carries only the
                    # single ACT self-wait from dead3's em(w-2) read.
                    obs2 = mpool.tile([Q, 1], BF16, tag="obs2", bufs=4)
                    i_obs2 = nc.scalar.copy(obs2, s16_v[:, 79, i0 + cw - 1 : i0 + cw])
                    if act_prev is not None:
                        add_dep_helper(i_obs2.ins, act_prev.ins, sync=False)
                    act_prev = i_obs2
                if k == nchunk - 1 and w + 2 < W:
                    # Prefetch window w+2 into the s_t buffer the STTs above
                    # just finished reading.  DMAs can only issue from
                    # SP/ACT/gpsimd; use ACT with a 1-element observer copy
                    # of the last STT's output so the issue carries no waits
                    # (the WAR on the STTs and the WAW on the old transfer
                    # are then both already in ACT's clock).
                    obs = mpool.tile([Q, 1], BF16, tag="obs", bufs=2)
                    i_obs = nc.scalar.copy(obs, s16_v[:, 79, cw - 1 : cw])
                    s_next = spool.tile(
                        [Q, NPROT * FI], F32, tag="s", name=f"s_t{w+2}"
                    )
                    i_dma = nc.scalar.dma_start(
                        out=s_next.rearrange("p (n i) -> p n i", n=NPROT),
                        in_=dview[:, :, w + 2, :].transpose([1, 0, 2]),
                    )
                    add_dep_helper(i_dma.ins, i_obs.ins, sync=False)
                    s_tiles.append(s_next)
                    act_prev = i_dma

                # em = exp(s16), bf16
                i_exp = nc.scalar.activation(
                    em_v[:, :, i0 : i0 + cw],
                    s16_v[:, :NPROT, i0 : i0 + cw],
                    mybir.ActivationFunctionType.Exp,
                )
                if act_prev is not None:
                    add_dep_helper(i_exp.ins, act_prev.ins, sync=False)

                for i in range(i0, i0 + cw):
                    nc.tensor.matmul(
                        g_ps,
                        s16_v[:, :, i],
                        em_v[:, :, i],
                        start=first,
                        stop=(w == W - 1 and i == FI - 1),
                    )
                    first = False

            if w + 2 < W:
                # Read the accumulator right after this window's last matmul:
                # the copy waits exactly on "PE done with window w", putting
                # that tick into ACT's clock for window w+2's exp.
                dead4 = mpool.tile([1, 1], F32, tag="dead4", bufs=2)
                dead4_insts.append(nc.scalar.copy(dead4, g_ps[0:1, 0:1]))
                dead4_tiles.append(dead4)

        # Absorb the ACT-PSUM-read serialization into DVE so the final
        # PSUM->SBUF copy carries only the PE wait.
        deadf = mpool.tile([1, 1], F32, tag="deadf", bufs=1)
        i_deadf = nc.vector.tensor_copy(deadf, dead4_tiles[-1])
        g_sb = singles.tile([NSLOT, NPROT], F32)
        i_gcopy = nc.vector.tensor_copy(g_sb, g_ps)
        add_dep_helper(i_gcopy.ins, i_deadf.ins, sync=False)
        nc.sync.dma_start(out=g_out[:, :], in_=g_sb)

    # The kernel-tail drain aggregates every outstanding semaphore into one
    # instruction; the CTRL struct cannot hold that many waits.  Split it
    # into a chain of single-wait drains.
    import copy as _copy

    for fn in nc.m.functions:
        for blk in fn.blocks:
            insts = blk.instructions
            for idx, ins in enumerate(list(insts)):
                si = ins.sync_info
                if type(ins).__name__ == "InstDrain" and si and len(si.on_wait) > 1:
                    waits = list(si.on_wait)
                    si.on_wait = waits[-1:]
                    pos = insts.index(ins)
                    for k, wt in enumerate(waits[:-1]):
                        d2 = _copy.deepcopy(ins)
                        d2.name = f"{ins.name}-split{k}"
                        d2.sync_info = type(si)(on_wait=[wt], on_update=[])
                        insts.insert(pos + k, d2)
                    break

    return nc


def _get_nc():
    if "nc" not in _NC_CACHE:
        _NC_CACHE["nc"] = build_nc()
    return _NC_CACHE["nc"]


def run_device(dist8, labf8, trace=False):
    """dist8: [8, 80, P] f32 class-major proto order; labf8: [8, P] f32 labels-1."""
    nc = _get_nc()
    cls = np.broadcast_to(np.arange(C, dtype=np.float32)[None, :], (Q, C))
    in_maps = []
    for b in range(B):
        labcls = np.concatenate([labf8[b].reshape(Q, 512), cls], axis=1)
        in_maps.append(
            {"dist": dist8[b], "labcls": np.ascontiguousarray(labcls)}
        )
    return run_bass_kernel_spmd(nc, in_maps, list(range(B)), trace=trace)


def kernel(
    prototype_distances,
    target_labels,
    proto_class,
    pair_i,
    pair_j,
    pair_cls,
    _trace=False,
    _results_out=None,
):
    dist = np.asarray(prototype_distances, dtype=np.float32).reshape(B, NPROT, P)
    labels = np.asarray(target_labels).reshape(B, P).astype(np.int64)
    proto_class = np.asarray(proto_class, dtype=np.int64)
    pair_i = np.asarray(pair_i, dtype=np.int64)
    pair_j = np.asarray(pair_j, dtype=np.int64)
    pair_cls = np.asarray(pair_cls, dtype=np.int64)

    # Permute prototypes to class-major layout: slot n -> class n // 8.
    perm = np.empty(NPROT, dtype=np.int64)
    for c in range(C):
        protos = np.nonzero(proto_class == c)[0]
        assert len(protos) == 8, "expect 8 prototypes per class"
        perm[8 * c : 8 * c + 8] = protos
    inv = np.empty(NPROT, dtype=np.int64)
    inv[perm] = np.arange(NPROT)

    if np.array_equal(perm, np.arange(NPROT)):
        dist_p = dist
    else:
        dist_p = np.ascontiguousarray(dist[:, perm, :])
    labf = np.ascontiguousarray((labels - 1).astype(np.float32))

    br = run_device(dist_p, labf, trace=_trace)
    if _results_out is not None:
        _results_out.append(br)

    total_vals = np.float64(0.0)
    total_valid = 0
    for b in range(B):
        out = br.results[b]["g"]  # [81, 80]; out[j, a] = G[a, j], out[80, a] = Z_a
        Z = out[NPROT].astype(np.float64)
        Gt = out[:NPROT].astype(np.float64)  # Gt[j, a] = sum_p em_a * s_j
        with np.errstate(divide="ignore", invalid="ignore"):
            A = np.where(Z[None, :] != 0.0, Gt / Z[None, :], 0.0)  # A[j, a] = E_a[d_j]
        lb = labels[b] - 1
        cnt = np.bincount(lb[lb >= 0], minlength=C)
        ii = inv[pair_i]
        jj = inv[pair_j]
        # A[x, a] = expectation of d_x under softmax of proto a
        kld = 0.5 * (A[jj, jj] - A[jj, ii] + A[ii, ii] - A[ii, jj])
        valid = cnt[pair_cls] >= 2
        total_vals += np.exp(-kld[valid]).sum()
        total_valid += int(valid.sum())

    if total_valid > 0:
        res = np.float32(total_vals / max(total_valid, 1))
    else:
        res = np.float32(0.0)
    return res


if __name__ == "__main__":
    rng = np.random.default_rng(0)
    d = rng.standard_normal((B, NPROT, 256, 256), dtype=np.float32)
    l = rng.integers(0, 11, (B, 256, 256))
    pc = (np.arange(NPROT) % 40) // 4
    pairs = []
    for s in range(2):
        for c in range(C):
            base = s * 40 + c * 4
            for a in range(4):
                for b2 in range(a + 1, 4):
                    pairs.append((base + a, base + b2, c))
    pairs = np.asarray(pairs, np.int32)
    print(kernel(d, l, pc, pairs[:, 0], pairs[:, 1], pairs[:, 2]))


# revision 23
# speedup vs baseline: 1.9259x; 1.1680x over previous
"""Trainium2 Bass kernel for nn_KLDLoss_18769007083961.

Math reformulation (validated vs reference, rel err ~3e-5 with bf16):
  For each image b, prototype a with class c(a), define over pixels p:
    s_a[p]  = d_a[p] + (label[p] != c(a)) * (-1e4)      # masked-biased distance
    em_a[p] = exp(s_a[p])                               # exactly 0 off-class (underflow)
    Z_a     = sum_p em_a[p]
    G[a,j]  = sum_p em_a[p] * s_j[p]   (j in same group => same class mask)
    A[a,j]  = G[a,j] / Z_a
  Symmetric KL for a same-group pair (i,j) (log-partition terms cancel):
    kld = 0.5 * (A[j,j] - A[j,i] + A[i,i] - A[i,j])
  loss = mean over valid pairs (class count >= 2) of exp(-kld).

Only same-class G entries are consumed, and em is exactly zero off-class,
so the bf16 rounding of the -1e4 bias in s never reaches the result: the
biased s tile can be written once in bf16 and feed both the exp (ACT) and
the matmul lhsT (PE).  bf16 matmul runs at 1 cycle/row vs fp32's 4.

Device kernel (one image per NeuronCore, 8 cores):
  Layout: pixel p = 512*q + 128*w + i  (q = SBUF partition, w = window, i = inner).
  Protos are permuted host-side to class-major order (slot = 8c + 4s + m) so the
  class bias applies to 8 contiguous protos per DVE op.
  Per window: DMA dist -> s_t[128, 80*128] f32; DVE builds (label != c) and
  writes s16[128, 81*128] bf16 = d + mne*-1e4 (slot 80 memset to 1.0);
  ACT computes em = exp(s16) bf16; 128 matmuls (lhsT = s16-slice [128,81],
  rhs = em-slice [128,80]) accumulate out[j,a] = G[a,j], row 80 = Z, in
  PSUM [81,80] f32.  The last window runs in 4 column chunks to shorten the
  pipeline tail after the final DMA.  Host does the tiny 120-pair combine.
"""

import sys
from contextlib import ExitStack

import numpy as np

sys.path.insert(0, "/opt/trn_rl_repo")

import concourse.bass as bass
import concourse.tile as tile
from concourse import mybir
from concourse.bass_utils import run_bass_kernel_spmd
from concourse.tile import add_dep_helper

B = 8
C = 10
NPROT = 80
NSLOT = NPROT + 1  # 80 protos + ones column for Z
P = 65536
Q = 128          # partitions = coarse pixel blocks of 512
W = 4            # windows per image
FI = 128         # inner pixels per window per partition
F32 = mybir.dt.float32
BF16 = mybir.dt.bfloat16

_NC_CACHE = {}


def build_nc():
    nc = bass.Bass()
    # dist pre-transposed host-side to [w, q, n, i]: every window DMA is a
    # plain contiguous copy (40KB per partition line, no descriptor storm)
    d_in = nc.dram_tensor("dist", [W * Q, NPROT * FI], F32, kind="ExternalInput")
    # labels [q, 512] packed with the 10 class constants -> cols 512..521
    lab_in = nc.dram_tensor("labcls", [Q, 512 + C], F32, kind="ExternalInput")
    g_out = nc.dram_tensor("g", [NSLOT, NPROT], F32, kind="ExternalOutput")

    with ExitStack() as ctx:
        tc = ctx.enter_context(tile.TileContext(nc))
        singles = ctx.enter_context(tc.tile_pool(name="singles", bufs=1))
        spool = ctx.enter_context(tc.tile_pool(name="spool", bufs=2))
        s16pool = ctx.enter_context(tc.tile_pool(name="s16pool", bufs=2))
        empool = ctx.enter_context(tc.tile_pool(name="empool", bufs=2))
        mpool = ctx.enter_context(tc.tile_pool(name="mpool", bufs=2))
        psum = ctx.enter_context(tc.tile_pool(name="psum", bufs=1, space="PSUM"))

        labels_t = singles.tile([Q, 512 + C], F32)
        nc.sync.dma_start(out=labels_t, in_=lab_in[:, :])
        cls_t = labels_t[:, 512 : 512 + C]

        g_ps = psum.tile([NSLOT, NPROT], F32)

        s_tiles = []
        for w in range(2):
            s_w = spool.tile([Q, NPROT * FI], F32, tag="s", name=f"s_t{w}")
            nc.sync.dma_start(out=s_w, in_=d_in[w * Q : (w + 1) * Q, :])
            s_tiles.append(s_w)

        # constant source for the ACT-side absorber writes below
        zconst = singles.tile([Q, 1], BF16)
        nc.vector.memset(zconst, 0.0)

        # Engines have a single sync-wait slot per instruction.  Every
        # cross-engine dependency is therefore carried by a dedicated
        # 1-element absorber op, pinned ahead of its consumer with no-sync
        # dep edges so the scheduler keeps the elision-enabling order.
        first = True
        em_tiles = []
        dead4_tiles = []
        dead4_insts = []
        for w in range(W):
            s_t = s_tiles[w]
            s_v = s_t.rearrange("p (n i) -> p n i", n=NPROT)

            # mne[p, c, i] = (labels != c) as 1.0/0.0, bf16
            mne = mpool.tile([Q, C * FI], BF16, tag="mne")
            mne_v = mne.rearrange("p (c i) -> p c i", c=C)
            lab_w = labels_t[:, w * FI : (w + 1) * FI]
            nc.vector.tensor_tensor(
                mne_v,
                lab_w.unsqueeze(1).broadcast_to([Q, C, FI]),
                cls_t.unsqueeze(2).broadcast_to([Q, C, FI]),
                mybir.AluOpType.not_equal,
            )

            # DVE absorber chain: (1) dist-DMA completion for this window
            probe = mpool.tile([Q, 1], F32, tag="probe", bufs=4)
            i_probe = nc.vector.tensor_copy(probe, s_t[:, 0:1])
            dve_prev = i_probe
            if w >= 2:
                # (2) ACT finished exp(w-2) (read byte from its LAST chunk),
                # which read the s16 buffer the STTs below recycle
                probe2 = mpool.tile([Q, 1], BF16, tag="probe2", bufs=4)
                i_probe2 = nc.vector.tensor_copy(
                    probe2, em_tiles[w - 2][:, FI - 1 : FI]
                )
                add_dep_helper(i_probe2.ins, dve_prev.ins, sync=False)
                dve_prev = i_probe2

            s16 = s16pool.tile([Q, NSLOT * FI], BF16, tag="s16")
            s16_v = s16.rearrange("p (n i) -> p n i", n=NSLOT)
            em = empool.tile([Q, NPROT * FI], BF16, tag="em")
            em_v = em.rearrange("p (n i) -> p n i", n=NPROT)
            em_tiles.append(em)

            # ones column (slot 80) -> Z row of the gram.  For w >= 2 its
            # bytes were read by every LDW of window w-2, so this memset
            # carries exactly the "PE done with window w-2" wait that the
            # STTs would otherwise each need.
            i_memset = nc.vector.memset(s16_v[:, NPROT, :], 1.0)
            add_dep_helper(i_memset.ins, dve_prev.ins, sync=False)
            dve_prev = i_memset

            # ACT absorber: reading an old-em byte absorbs the same-engine
            # WAW tick (exp(w) overwrites exp(w-2)'s output); the PE tick was
            # absorbed by dead_act at the end of window w-2.
            act_abs = None
            if w >= 2:
                dead3 = mpool.tile([Q, 1], BF16, tag="dead3", bufs=2)
                act_abs = nc.scalar.copy(dead3, em_tiles[w - 2][:, 2:3])
                add_dep_helper(act_abs.ins, dead4_insts[w - 2].ins, sync=False)

            # All windows chunked so exp/PE start as soon as the first slice
            # of STTs lands; the last window uses finer chunks to shorten the
            # tail after the final DMA.
            nchunk = 4 if w == W - 1 else 2
            cw = FI // nchunk
            for k in range(nchunk):
                i0 = k * cw
                # s16 = (mne * -1e4) + d, bf16 out, 8 protos per class block
                for c in range(C):
                    n0 = 8 * c
                    mne_b = (
                        mne_v[:, c, i0 : i0 + cw]
                        .unsqueeze(1)
                        .broadcast_to([Q, 8, cw])
                    )
                    i_stt = nc.vector.scalar_tensor_tensor(
                        s16_v[:, n0 : n0 + 8, i0 : i0 + cw],
                        mne_b,
                        -1.0e4,
                        s_v[:, n0 : n0 + 8, i0 : i0 + cw],
                        mybir.AluOpType.mult,
                        mybir.AluOpType.add,
                    )
                    if k == 0 and c == 0:
                        add_dep_helper(i_stt.ins, dve_prev.ins, sync=False)

                act_prev = act_abs
                act_abs = None
                if w >= 2:
                    # ACT-side observer of the last STT of this chunk: exp
                    # below then sheds its DVE wait and carries only the
                    # single ACT self-wait from dead3's em(w-2) read.
                    obs2 = mpool.tile([Q, 1], BF16, tag="obs2", bufs=4)
                    i_obs2 = nc.scalar.copy(obs2, s16_v[:, 79, i0 + cw - 1 : i0 + cw])
                    if act_prev is not None:
                        add_dep_helper(i_obs2.ins, act_prev.ins, sync=False)
                    act_prev = i_obs2
                if k == nchunk - 1 and w + 2 < W:
                    # Prefetch window w+2 into the s_t buffer the STTs above
                    # just finished reading.  DMAs can only issue from
                    # SP/ACT/gpsimd; use ACT with a 1-element observer copy
                    # of the last STT's output so the issue carries no waits
                    # (the WAR on the STTs and the WAW on the old transfer
                    # are then both already in ACT's clock).
                    obs = mpool.tile([Q, 1], BF16, tag="obs", bufs=2)
                    i_obs = nc.scalar.copy(
                        obs, s16_v[:, 79, i0 + cw - 1 : i0 + cw]
                    )
                    s_next = spool.tile(
                        [Q, NPROT * FI], F32, tag="s", name=f"s_t{w+2}"
                    )
                    i_dma = nc.scalar.dma_start(
                        out=s_next, in_=d_in[(w + 2) * Q : (w + 3) * Q, :]
                    )
                    add_dep_helper(i_dma.ins, i_obs.ins, sync=False)
                    s_tiles.append(s_next)
                    act_prev = i_dma

                # em = exp(s16), bf16
                i_exp = nc.scalar.activation(
                    em_v[:, :, i0 : i0 + cw],
                    s16_v[:, :NPROT, i0 : i0 + cw],
                    mybir.ActivationFunctionType.Exp,
                )
                if act_prev is not None:
                    add_dep_helper(i_exp.ins, act_prev.ins, sync=False)

                for i in range(i0, i0 + cw):
                    nc.tensor.matmul(
                        g_ps,
                        s16_v[:, :, i],
                        em_v[:, :, i],
                        start=first,
                        stop=(w == W - 1 and i == FI - 1),
                    )
                    first = False

            if w + 2 < W:
                # Read the accumulator right after this window's last matmul:
                # the copy waits exactly on "PE done with window w", putting
                # that tick into ACT's clock for window w+2's exp.
                dead4 = mpool.tile([1, 1], F32, tag="dead4", bufs=2)
                dead4_insts.append(nc.scalar.copy(dead4, g_ps[0:1, 0:1]))
                dead4_tiles.append(dead4)

        # Absorb the ACT-PSUM-read serialization into DVE so the final
        # PSUM->SBUF copy carries only the PE wait.
        deadf = mpool.tile([1, 1], F32, tag="deadf", bufs=1)
        i_deadf = nc.vector.tensor_copy(deadf, dead4_tiles[-1])
        g_sb = singles.tile([NSLOT, NPROT], F32)
        i_gcopy = nc.vector.tensor_copy(g_sb, g_ps)
        add_dep_helper(i_gcopy.ins, i_deadf.ins, sync=False)
        nc.sync.dma_start(out=g_out[:, :], in_=g_sb)

    # The kernel-tail drain aggregates every outstanding semaphore into one
    # instruction; the CTRL struct cannot hold that many waits.  Split it
    # into a chain of single-wait drains.
    import copy as _copy

    for fn in nc.m.functions:
        for blk in fn.blocks:
            insts = blk.instructions
            for idx, ins in enumerate(list(insts)):
                si = ins.sync_info
                if type(ins).__name__ == "InstDrain" and si and len(si.on_wait) > 1:
                    waits = list(si.on_wait)
                    si.on_wait = waits[-1:]
                    pos = insts.index(ins)
                    for k, wt in enumerate(waits[:-1]):
                        d2 = _copy.deepcopy(ins)
                        d2.name = f"{ins.name}-split{k}"
                        d2.sync_info = type(si)(on_wait=[wt], on_update=[])
                        insts.insert(pos + k, d2)
                    break

    return nc


def _get_nc():
    if "nc" not in _NC_CACHE:
        _NC_CACHE["nc"] = build_nc()
    return _NC_CACHE["nc"]


def run_device(dist8, labf8, trace=False):
    """dist8: [8, W*Q, NPROT*FI] f32 device layout; labf8: [8, P] f32 labels-1."""
    nc = _get_nc()
    cls = np.broadcast_to(np.arange(C, dtype=np.float32)[None, :], (Q, C))
    in_maps = []
    for b in range(B):
        labcls = np.concatenate([labf8[b].reshape(Q, 512), cls], axis=1)
        in_maps.append(
            {"dist": dist8[b], "labcls": np.ascontiguousarray(labcls)}
        )
    return run_bass_kernel_spmd(nc, in_maps, list(range(B)), trace=trace)


def kernel(
    prototype_distances,
    target_labels,
    proto_class,
    pair_i,
    pair_j,
    pair_cls,
    _trace=False,
    _results_out=None,
):
    dist = np.asarray(prototype_distances, dtype=np.float32).reshape(B, NPROT, P)
    labels = np.asarray(target_labels).reshape(B, P).astype(np.int64)
    proto_class = np.asarray(proto_class, dtype=np.int64)
    pair_i = np.asarray(pair_i, dtype=np.int64)
    pair_j = np.asarray(pair_j, dtype=np.int64)
    pair_cls = np.asarray(pair_cls, dtype=np.int64)

    # Permute prototypes to class-major layout: slot n -> class n // 8.
    perm = np.empty(NPROT, dtype=np.int64)
    for c in range(C):
        protos = np.nonzero(proto_class == c)[0]
        assert len(protos) == 8, "expect 8 prototypes per class"
        perm[8 * c : 8 * c + 8] = protos
    inv = np.empty(NPROT, dtype=np.int64)
    inv[perm] = np.arange(NPROT)

    # Device layout [w, q, n, i]: pixel p = 512*q + 128*w + i, protos class-
    # major.  One transpose+copy host-side buys fully contiguous device DMAs.
    dist_v = dist[:, perm, :].reshape(B, NPROT, Q, W, FI)
    dist_p = np.ascontiguousarray(dist_v.transpose(0, 3, 2, 1, 4)).reshape(
        B, W * Q, NPROT * FI
    )
    labf = np.ascontiguousarray((labels - 1).astype(np.float32))

    br = run_device(dist_p, labf, trace=_trace)
    if _results_out is not None:
        _results_out.append(br)

    total_vals = np.float64(0.0)
    total_valid = 0
    for b in range(B):
        out = br.results[b]["g"]  # [81, 80]; out[j, a] = G[a, j], out[80, a] = Z_a
        Z = out[NPROT].astype(np.float64)
        Gt = out[:NPROT].astype(np.float64)  # Gt[j, a] = sum_p em_a * s_j
        with np.errstate(divide="ignore", invalid="ignore"):
            A = np.where(Z[None, :] != 0.0, Gt / Z[None, :], 0.0)  # A[j, a] = E_a[d_j]
        lb = labels[b] - 1
        cnt = np.bincount(lb[lb >= 0], minlength=C)
        ii = inv[pair_i]
        jj = inv[pair_j]
        # A[x, a] = expectation of d_x under softmax of proto a
        kld = 0.5 * (A[jj, jj] - A[jj, ii] + A[ii, ii] - A[ii, jj])
        valid = cnt[pair_cls] >= 2
        total_vals += np.exp(-kld[valid]).sum()
        total_valid += int(valid.sum())

    if total_valid > 0:
        res = np.float32(total_vals / max(total_valid, 1))
    else:
        res = np.float32(0.0)
    return res


if __name__ == "__main__":
    rng = np.random.default_rng(0)
    d = rng.standard_normal((B, NPROT, 256, 256), dtype=np.float32)
    l = rng.integers(0, 11, (B, 256, 256))
    pc = (np.arange(NPROT) % 40) // 4
    pairs = []
    for s in range(2):
        for c in range(C):
            base = s * 40 + c * 4
            for a in range(4):
                for b2 in range(a + 1, 4):
                    pairs.append((base + a, base + b2, c))
    pairs = np.asarray(pairs, np.int32)
    print(kernel(d, l, pc, pairs[:, 0], pairs[:, 1], pairs[:, 2]))


# revision 31
# speedup vs baseline: 2.1220x; 1.1018x over previous
"""Trainium2 Bass kernel for nn_KLDLoss_18769007083961.

Math reformulation (validated vs reference, rel err ~3e-5 with bf16):
  For each image b, prototype a with class c(a), define over pixels p:
    s_a[p]  = d_a[p] + (label[p] != c(a)) * (-1e4)      # masked-biased distance
    em_a[p] = exp(s_a[p])                               # exactly 0 off-class (underflow)
    Z_a     = sum_p em_a[p]
    G[a,j]  = sum_p em_a[p] * s_j[p]   (j in same group => same class mask)
    A[a,j]  = G[a,j] / Z_a
  Symmetric KL for a same-group pair (i,j) (log-partition terms cancel):
    kld = 0.5 * (A[j,j] - A[j,i] + A[i,i] - A[i,j])
  loss = mean over valid pairs (class count >= 2) of exp(-kld).

Only same-class G entries are consumed, and em is exactly zero off-class,
so the bf16 rounding of the -1e4 bias in s never reaches the result: the
biased s tile can be written once in bf16 and feed both the exp (ACT) and
the matmul lhsT (PE).  bf16 matmul runs at 1 cycle/row vs fp32's 4.

Device kernel (one image per NeuronCore, 8 cores):
  Layout: pixel p = 512*q + 128*w + i  (q = SBUF partition, w = window, i = inner).
  Protos are permuted host-side to class-major order (slot = 8c + 4s + m) so the
  class bias applies to 8 contiguous protos per DVE op.
  Per window: DMA dist -> s_t[128, 80*128] f32; DVE builds (label != c) and
  writes s16[128, 81*128] bf16 = d + mne*-1e4 (slot 80 memset to 1.0);
  ACT computes em = exp(s16) bf16; 128 matmuls (lhsT = s16-slice [128,81],
  rhs = em-slice [128,80]) accumulate out[j,a] = G[a,j], row 80 = Z, in
  PSUM [81,80] f32.  The last window runs in 4 column chunks to shorten the
  pipeline tail after the final DMA.  Host does the tiny 120-pair combine.
"""

import sys
from contextlib import ExitStack

import numpy as np

sys.path.insert(0, "/opt/trn_rl_repo")

import concourse.bass as bass
import concourse.tile as tile
from concourse import mybir
from concourse.bass_utils import run_bass_kernel_spmd
from concourse.tile import add_dep_helper

B = 8
C = 10
NPROT = 80
NSLOT = NPROT + 1  # 80 protos + ones column for Z
P = 65536
Q = 128          # partitions = coarse pixel blocks of 512
W = 4            # windows per image
FI = 128         # inner pixels per window per partition
F32 = mybir.dt.float32
BF16 = mybir.dt.bfloat16

_NC_CACHE = {}


def build_nc():
    nc = bass.Bass()
    # dist pre-transposed host-side to [w, half, q, n, i]: every half-window
    # DMA is a plain contiguous copy, so the first STT chunk can start after
    # ~1/8 of the image has landed instead of waiting for a full window.
    d_in = nc.dram_tensor(
        "dist", [W * 2 * Q, NPROT * (FI // 2)], F32, kind="ExternalInput"
    )
    # labels [q, 512] packed with the 10 class constants -> cols 512..521
    lab_in = nc.dram_tensor("labcls", [Q, 512 + C], F32, kind="ExternalInput")
    g_out = nc.dram_tensor("g", [NSLOT, NPROT], F32, kind="ExternalOutput")

    with ExitStack() as ctx:
        tc = ctx.enter_context(tile.TileContext(nc))
        singles = ctx.enter_context(tc.tile_pool(name="singles", bufs=1))
        spool = ctx.enter_context(tc.tile_pool(name="spool", bufs=2))
        s16pool = ctx.enter_context(tc.tile_pool(name="s16pool", bufs=2))
        empool = ctx.enter_context(tc.tile_pool(name="empool", bufs=2))
        mpool = ctx.enter_context(tc.tile_pool(name="mpool", bufs=2))
        psum = ctx.enter_context(tc.tile_pool(name="psum", bufs=1, space="PSUM"))

        labels_t = singles.tile([Q, 512 + C], F32)
        nc.sync.dma_start(out=labels_t, in_=lab_in[:, :])
        cls_t = labels_t[:, 512 : 512 + C]

        g_ps = psum.tile([NSLOT, NPROT], F32)

        HW_ = FI // 2  # pixels per half-window
        s_tiles = []
        for w in range(2):
            s_w = spool.tile([Q, NPROT * FI], F32, tag="s", name=f"s_t{w}")
            for h in range(2):
                nc.sync.dma_start(
                    out=s_w[:, h * NPROT * HW_ : (h + 1) * NPROT * HW_],
                    in_=d_in[(2 * w + h) * Q : (2 * w + h + 1) * Q, :],
                )
            s_tiles.append(s_w)

        # constant source for the ACT-side absorber writes below
        zconst = singles.tile([Q, 1], BF16)
        nc.vector.memset(zconst, 0.0)

        # Engines have a single sync-wait slot per instruction.  Every
        # cross-engine dependency is therefore carried by a dedicated
        # 1-element absorber op, pinned ahead of its consumer with no-sync
        # dep edges so the scheduler keeps the elision-enabling order.
        first = True
        em_tiles = []
        dead4_tiles = []
        dead4_insts = []
        for w in range(W):
            s_t = s_tiles[w]
            s_v = s_t.rearrange("p (n i) -> p n i", n=NPROT)

            # mne[p, c, i] = (labels != c) as 1.0/0.0, bf16
            mne = mpool.tile([Q, C * FI], BF16, tag="mne")
            mne_v = mne.rearrange("p (c i) -> p c i", c=C)
            lab_w = labels_t[:, w * FI : (w + 1) * FI]
            nc.vector.tensor_tensor(
                mne_v,
                lab_w.unsqueeze(1).broadcast_to([Q, C, FI]),
                cls_t.unsqueeze(2).broadcast_to([Q, C, FI]),
                mybir.AluOpType.not_equal,
            )

            # DVE absorber chain: (1) dist-DMA completion for this window
            probe = mpool.tile([Q, 1], F32, tag="probe", bufs=4)
            i_probe = nc.vector.tensor_copy(probe, s_t[:, 0:1])
            dve_prev = i_probe
            if w >= 2:
                # (2) ACT finished exp(w-2) (read byte from its LAST chunk),
                # which read the s16 buffer the STTs below recycle
                probe2 = mpool.tile([Q, 1], BF16, tag="probe2", bufs=4)
                i_probe2 = nc.vector.tensor_copy(
                    probe2, em_tiles[w - 2][:, FI - 1 : FI]
                )
                add_dep_helper(i_probe2.ins, dve_prev.ins, sync=False)
                dve_prev = i_probe2

            s16 = s16pool.tile([Q, NSLOT * FI], BF16, tag="s16")
            s16_v = s16.rearrange("p (n i) -> p n i", n=NSLOT)
            em = empool.tile([Q, NPROT * FI], BF16, tag="em")
            em_v = em.rearrange("p (n i) -> p n i", n=NPROT)
            em_tiles.append(em)

            # ones column (slot 80) -> Z row of the gram.  For w >= 2 its
            # bytes were read by every LDW of window w-2, so this memset
            # carries exactly the "PE done with window w-2" wait that the
            # STTs would otherwise each need.
            i_memset = nc.vector.memset(s16_v[:, NPROT, :], 1.0)
            add_dep_helper(i_memset.ins, dve_prev.ins, sync=False)
            dve_prev = i_memset

            # ACT absorber: reading an old-em byte absorbs the same-engine
            # WAW tick (exp(w) overwrites exp(w-2)'s output); the PE tick was
            # absorbed by dead_act at the end of window w-2.
            act_abs = None
            if w >= 2:
                # Read a byte exp(w-2)'s LAST chunk wrote: the single wait
                # "ACT >= exp(w-2, k3)" dominates every chunk's WAW below.
                dead3 = mpool.tile([Q, 1], BF16, tag="dead3", bufs=2)
                act_abs = nc.scalar.copy(
                    dead3, em_tiles[w - 2][:, FI - 2 : FI - 1]
                )
                add_dep_helper(act_abs.ins, dead4_insts[w - 2].ins, sync=False)

            # 4 chunks per window: exp/PE start as soon as the first quarter
            # of STTs lands, and the tail after the last DMA stays short.
            # s_t arrives as two contiguous halves [half, n, HW_].
            s_v5 = s_t.rearrange("p (h n i) -> p h n i", h=2, n=NPROT)
            nchunk = 4
            cw = FI // nchunk
            s_next = None
            for k in range(nchunk):
                i0 = k * cw
                h = k // 2          # which DMA half this chunk reads
                hi = (k % 2) * cw   # offset within the half
                # s16 = (mne * -1e4) + d, bf16 out, 8 protos per class block
                for c in range(C):
                    n0 = 8 * c
                    mne_b = (
                        mne_v[:, c, i0 : i0 + cw]
                        .unsqueeze(1)
                        .broadcast_to([Q, 8, cw])
                    )
                    i_stt = nc.vector.scalar_tensor_tensor(
                        s16_v[:, n0 : n0 + 8, i0 : i0 + cw],
                        mne_b,
                        -1.0e4,
                        s_v5[:, h, n0 : n0 + 8, hi : hi + cw],
                        mybir.AluOpType.mult,
                        mybir.AluOpType.add,
                    )
                    if k == 0 and c == 0:
                        add_dep_helper(i_stt.ins, dve_prev.ins, sync=False)

                act_prev = act_abs
                act_abs = None
                # ACT-side observer of the last STT of this chunk: the exp
                # below then sheds its DVE wait, and the prefetch DMA can
                # issue waitlessly right here.
                obs2 = mpool.tile([Q, 1], BF16, tag="obs2", bufs=8)
                i_obs2 = nc.scalar.copy(
                    obs2, s16_v[:, 79, i0 + cw - 1 : i0 + cw]
                )
                if act_prev is not None:
                    add_dep_helper(i_obs2.ins, act_prev.ins, sync=False)
                act_prev = i_obs2
                if k == nchunk - 1 and w + 2 < W:
                    # All STTs of this window are done; prefetch both halves
                    # of window w+2 into the freed buffer.  The two 1-elem
                    # copies absorb the old transfers' DMAHW ticks so the
                    # dma issues carry no waits (the DVE tick came via obs2).
                    s_next = spool.tile(
                        [Q, NPROT * FI], F32, tag="s", name=f"s_t{w+2}"
                    )
                    s_tiles.append(s_next)
                    prev = i_obs2
                    for h2 in range(2):
                        dmaobs = mpool.tile([Q, 1], F32, tag="dmaobs", bufs=4)
                        i_do = nc.scalar.copy(
                            dmaobs, s_t[:, h2 * NPROT * HW_ : h2 * NPROT * HW_ + 1]
                        )
                        add_dep_helper(i_do.ins, prev.ins, sync=False)
                        prev = i_do
                    for h2 in range(2):
                        i_dma = nc.scalar.dma_start(
                            out=s_next[
                                :, h2 * NPROT * HW_ : (h2 + 1) * NPROT * HW_
                            ],
                            in_=d_in[
                                (2 * (w + 2) + h2) * Q : (2 * (w + 2) + h2 + 1)
                                * Q,
                                :,
                            ],
                        )
                        add_dep_helper(i_dma.ins, prev.ins, sync=False)
                        prev = i_dma
                    act_prev = prev

                # em = exp(s16), bf16
                i_exp = nc.scalar.activation(
                    em_v[:, :, i0 : i0 + cw],
                    s16_v[:, :NPROT, i0 : i0 + cw],
                    mybir.ActivationFunctionType.Exp,
                )
                if act_prev is not None:
                    add_dep_helper(i_exp.ins, act_prev.ins, sync=False)

                for i in range(i0, i0 + cw):
                    nc.tensor.matmul(
                        g_ps,
                        s16_v[:, :, i],
                        em_v[:, :, i],
                        start=first,
                        stop=(w == W - 1 and i == FI - 1),
                    )
                    first = False

            if w + 2 < W:
                # Read the accumulator right after this window's last matmul:
                # the copy waits exactly on "PE done with window w", putting
                # that tick into ACT's clock for window w+2's exp.
                dead4 = mpool.tile([1, 1], F32, tag="dead4", bufs=2)
                dead4_insts.append(nc.scalar.copy(dead4, g_ps[0:1, 0:1]))
                dead4_tiles.append(dead4)

        # Absorb the ACT-PSUM-read serialization into DVE so the final
        # PSUM->SBUF copy carries only the PE wait.
        deadf = mpool.tile([1, 1], F32, tag="deadf", bufs=1)
        i_deadf = nc.vector.tensor_copy(deadf, dead4_tiles[-1])
        g_sb = singles.tile([NSLOT, NPROT], F32)
        i_gcopy = nc.vector.tensor_copy(g_sb, g_ps)
        add_dep_helper(i_gcopy.ins, i_deadf.ins, sync=False)
        # Output DMA from ACT behind a g_sb observer, so the issue carries
        # at most the DMAHW semaphore-recycling wait.
        gobs = mpool.tile([1, 1], F32, tag="gobs", bufs=1)
        i_gobs = nc.scalar.copy(gobs, g_sb[0:1, 0:1])
        add_dep_helper(i_gobs.ins, i_gcopy.ins, sync=False)
        i_gdma = nc.scalar.dma_start(out=g_out[:, :], in_=g_sb)
        add_dep_helper(i_gdma.ins, i_gobs.ins, sync=False)

    # The kernel-tail drain aggregates every outstanding semaphore into one
    # instruction; the CTRL struct cannot hold that many waits.  Split it
    # into a chain of single-wait drains.
    import copy as _copy

    for fn in nc.m.functions:
        for blk in fn.blocks:
            insts = blk.instructions
            for idx, ins in enumerate(list(insts)):
                si = ins.sync_info
                if type(ins).__name__ == "InstDrain" and si and len(si.on_wait) > 1:
                    waits = list(si.on_wait)
                    si.on_wait = waits[-1:]
                    pos = insts.index(ins)
                    for k, wt in enumerate(waits[:-1]):
                        d2 = _copy.deepcopy(ins)
                        d2.name = f"{ins.name}-split{k}"
                        d2.sync_info = type(si)(on_wait=[wt], on_update=[])
                        insts.insert(pos + k, d2)
                    break

    return nc


def _get_nc():
    if "nc" not in _NC_CACHE:
        _NC_CACHE["nc"] = build_nc()
    return _NC_CACHE["nc"]


def run_device(dist8, labf8, trace=False):
    """dist8: [8, W*Q, NPROT*FI] f32 device layout; labf8: [8, P] f32 labels-1."""
    nc = _get_nc()
    cls = np.broadcast_to(np.arange(C, dtype=np.float32)[None, :], (Q, C))
    in_maps = []
    for b in range(B):
        labcls = np.concatenate([labf8[b].reshape(Q, 512), cls], axis=1)
        in_maps.append(
            {"dist": dist8[b], "labcls": np.ascontiguousarray(labcls)}
        )
    return run_bass_kernel_spmd(nc, in_maps, list(range(B)), trace=trace)


def kernel(
    prototype_distances,
    target_labels,
    proto_class,
    pair_i,
    pair_j,
    pair_cls,
    _trace=False,
    _results_out=None,
):
    dist = np.asarray(prototype_distances, dtype=np.float32).reshape(B, NPROT, P)
    labels = np.asarray(target_labels).reshape(B, P).astype(np.int64)
    proto_class = np.asarray(proto_class, dtype=np.int64)
    pair_i = np.asarray(pair_i, dtype=np.int64)
    pair_j = np.asarray(pair_j, dtype=np.int64)
    pair_cls = np.asarray(pair_cls, dtype=np.int64)

    # Permute prototypes to class-major layout: slot n -> class n // 8.
    perm = np.empty(NPROT, dtype=np.int64)
    for c in range(C):
        protos = np.nonzero(proto_class == c)[0]
        assert len(protos) == 8, "expect 8 prototypes per class"
        perm[8 * c : 8 * c + 8] = protos
    inv = np.empty(NPROT, dtype=np.int64)
    inv[perm] = np.arange(NPROT)

    # Device layout [w, half, q, n, i]: pixel p = 512*q + 128*w + 64*h + i,
    # protos class-major.  One transpose+copy host-side buys fully
    # contiguous half-window device DMAs.
    HW_ = FI // 2
    dist_v = dist[:, perm, :].reshape(B, NPROT, Q, W, 2, HW_)
    dist_p = np.ascontiguousarray(dist_v.transpose(0, 3, 4, 2, 1, 5)).reshape(
        B, W * 2 * Q, NPROT * HW_
    )
    labf = np.ascontiguousarray((labels - 1).astype(np.float32))

    br = run_device(dist_p, labf, trace=_trace)
    if _results_out is not None:
        _results_out.append(br)

    total_vals = np.float64(0.0)
    total_valid = 0
    for b in range(B):
        out = br.results[b]["g"]  # [81, 80]; out[j, a] = G[a, j], out[80, a] = Z_a
        Z = out[NPROT].astype(np.float64)
        Gt = out[:NPROT].astype(np.float64)  # Gt[j, a] = sum_p em_a * s_j
        with np.errstate(divide="ignore", invalid="ignore"):
            A = np.where(Z[None, :] != 0.0, Gt / Z[None, :], 0.0)  # A[j, a] = E_a[d_j]
        lb = labels[b] - 1
        cnt = np.bincount(lb[lb >= 0], minlength=C)
        ii = inv[pair_i]
        jj = inv[pair_j]
        # A[x, a] = expectation of d_x under softmax of proto a
        kld = 0.5 * (A[jj, jj] - A[jj, ii] + A[ii, ii] - A[ii, jj])
        valid = cnt[pair_cls] >= 2
        total_vals += np.exp(-kld[valid]).sum()
        total_valid += int(valid.sum())

    if total_valid > 0:
        res = np.float32(total_vals / max(total_valid, 1))
    else:
        res = np.float32(0.0)
    return res


if __name__ == "__main__":
    rng = np.random.default_rng(0)
    d = rng.standard_normal((B, NPROT, 256, 256), dtype=np.float32)
    l = rng.integers(0, 11, (B, 256, 256))
    pc = (np.arange(NPROT) % 40) // 4
    pairs = []
    for s in range(2):
        for c in range(C):
            base = s * 40 + c * 4
            for a in range(4):
                for b2 in range(a + 1, 4):
                    pairs.append((base + a, base + b2, c))
    pairs = np.asarray(pairs, np.int32)
    print(kernel(d, l, pc, pairs[:, 0], pairs[:, 1], pairs[:, 2]))


# revision 37
# speedup vs baseline: 2.1997x; 1.0366x over previous
"""Trainium2 Bass kernel for nn_KLDLoss_18769007083961.

Math reformulation (validated vs reference, rel err ~3e-5 with bf16):
  For each image b, prototype a with class c(a), define over pixels p:
    s_a[p]  = d_a[p] + (label[p] != c(a)) * (-1e4)      # masked-biased distance
    em_a[p] = exp(s_a[p])                               # exactly 0 off-class (underflow)
    Z_a     = sum_p em_a[p]
    G[a,j]  = sum_p em_a[p] * s_j[p]   (j in same group => same class mask)
    A[a,j]  = G[a,j] / Z_a
  Symmetric KL for a same-group pair (i,j) (log-partition terms cancel):
    kld = 0.5 * (A[j,j] - A[j,i] + A[i,i] - A[i,j])
  loss = mean over valid pairs (class count >= 2) of exp(-kld).

Only same-class G entries are consumed, and em is exactly zero off-class,
so the bf16 rounding of the -1e4 bias in s never reaches the result: the
biased s tile can be written once in bf16 and feed both the exp (ACT) and
the matmul lhsT (PE).  bf16 matmul runs at 1 cycle/row vs fp32's 4.

Device kernel (one image per NeuronCore, 8 cores):
  Layout: pixel p = 512*q + 128*w + i  (q = SBUF partition, w = window, i = inner).
  Protos are permuted host-side to class-major order (slot = 8c + 4s + m) so the
  class bias applies to 8 contiguous protos per DVE op.
  Per window: DMA dist -> s_t[128, 80*128] f32; DVE builds (label != c) and
  writes s16[128, 81*128] bf16 = d + mne*-1e4 (slot 80 memset to 1.0);
  ACT computes em = exp(s16) bf16; 128 matmuls (lhsT = s16-slice [128,81],
  rhs = em-slice [128,80]) accumulate out[j,a] = G[a,j], row 80 = Z, in
  PSUM [81,80] f32.  The last window runs in 4 column chunks to shorten the
  pipeline tail after the final DMA.  Host does the tiny 120-pair combine.
"""

import sys
from contextlib import ExitStack

import numpy as np

sys.path.insert(0, "/opt/trn_rl_repo")

import concourse.bass as bass
import concourse.tile as tile
from concourse import mybir
from concourse.bass_utils import run_bass_kernel_spmd
from concourse.tile import add_dep_helper

B = 8
C = 10
NPROT = 80
NSLOT = NPROT + 1  # 80 protos + ones column for Z
P = 65536
Q = 128          # partitions = coarse pixel blocks of 512
W = 4            # windows per image
FI = 128         # inner pixels per window per partition
F32 = mybir.dt.float32
BF16 = mybir.dt.bfloat16

_NC_CACHE = {}


def build_nc():
    nc = bass.Bass()
    # dist pre-transposed host-side to [w, quarter, q, n, i]: every quarter-
    # window DMA is a plain contiguous copy, so the first STT chunk starts
    # after ~1/16 of the image has landed.
    d_in = nc.dram_tensor(
        "dist", [W * 4 * Q, NPROT * (FI // 4)], F32, kind="ExternalInput"
    )
    # labels [q, 512] packed with the 10 class constants -> cols 512..521
    lab_in = nc.dram_tensor("labcls", [Q, 512 + C], F32, kind="ExternalInput")
    g_out = nc.dram_tensor("g", [NSLOT, NPROT], F32, kind="ExternalOutput")

    with ExitStack() as ctx:
        tc = ctx.enter_context(tile.TileContext(nc))
        singles = ctx.enter_context(tc.tile_pool(name="singles", bufs=1))
        spool = ctx.enter_context(tc.tile_pool(name="spool", bufs=2))
        s16pool = ctx.enter_context(tc.tile_pool(name="s16pool", bufs=2))
        empool = ctx.enter_context(tc.tile_pool(name="empool", bufs=2))
        mpool = ctx.enter_context(tc.tile_pool(name="mpool", bufs=2))
        psum = ctx.enter_context(tc.tile_pool(name="psum", bufs=1, space="PSUM"))

        labels_t = singles.tile([Q, 512 + C], F32)
        nc.sync.dma_start(out=labels_t, in_=lab_in[:, :])
        cls_t = labels_t[:, 512 : 512 + C]

        g_ps = psum.tile([NSLOT, NPROT], F32)

        QW_ = FI // 4  # pixels per quarter-window
        QB = NPROT * QW_  # sbuf columns per quarter block
        s_tiles = []
        for w in range(2):
            s_w = spool.tile([Q, NPROT * FI], F32, tag="s", name=f"s_t{w}")
            for k in range(4):
                nc.sync.dma_start(
                    out=s_w[:, k * QB : (k + 1) * QB],
                    in_=d_in[(4 * w + k) * Q : (4 * w + k + 1) * Q, :],
                )
            s_tiles.append(s_w)

        # constant source for the ACT-side absorber writes below
        zconst = singles.tile([Q, 1], BF16)
        nc.vector.memset(zconst, 0.0)

        # Engines have a single sync-wait slot per instruction.  Every
        # cross-engine dependency is therefore carried by a dedicated
        # 1-element absorber op, pinned ahead of its consumer with no-sync
        # dep edges so the scheduler keeps the elision-enabling order.
        first = True
        em_tiles = []
        dead4_tiles = []
        dead4_insts = []
        for w in range(W):
            s_t = s_tiles[w]
            s_v = s_t.rearrange("p (n i) -> p n i", n=NPROT)

            # mne[p, c, i] = (labels != c) as 1.0/0.0, bf16
            mne = mpool.tile([Q, C * FI], BF16, tag="mne")
            mne_v = mne.rearrange("p (c i) -> p c i", c=C)
            lab_w = labels_t[:, w * FI : (w + 1) * FI]
            nc.vector.tensor_tensor(
                mne_v,
                lab_w.unsqueeze(1).broadcast_to([Q, C, FI]),
                cls_t.unsqueeze(2).broadcast_to([Q, C, FI]),
                mybir.AluOpType.not_equal,
            )

            # DVE absorber chain: (1) dist-DMA completion for this window
            probe = mpool.tile([Q, 1], F32, tag="probe", bufs=4)
            i_probe = nc.vector.tensor_copy(probe, s_t[:, 0:1])
            dve_prev = i_probe
            if w >= 2:
                # (2) ACT finished exp(w-2) (read byte from its LAST chunk),
                # which read the s16 buffer the STTs below recycle
                probe2 = mpool.tile([Q, 1], BF16, tag="probe2", bufs=4)
                i_probe2 = nc.vector.tensor_copy(
                    probe2, em_tiles[w - 2][:, FI - 1 : FI]
                )
                add_dep_helper(i_probe2.ins, dve_prev.ins, sync=False)
                dve_prev = i_probe2

            s16 = s16pool.tile([Q, NSLOT * FI], BF16, tag="s16")
            s16_v = s16.rearrange("p (n i) -> p n i", n=NSLOT)
            em = empool.tile([Q, NPROT * FI], BF16, tag="em")
            em_v = em.rearrange("p (n i) -> p n i", n=NPROT)
            em_tiles.append(em)

            # ones column (slot 80) -> Z row of the gram.  For w >= 2 its
            # bytes were read by every LDW of window w-2, so this memset
            # carries exactly the "PE done with window w-2" wait that the
            # STTs would otherwise each need.
            i_memset = nc.vector.memset(s16_v[:, NPROT, :], 1.0)
            add_dep_helper(i_memset.ins, dve_prev.ins, sync=False)
            dve_prev = i_memset

            # ACT absorber: reading an old-em byte absorbs the same-engine
            # WAW tick (exp(w) overwrites exp(w-2)'s output); the PE tick was
            # absorbed by dead_act at the end of window w-2.
            act_abs = None
            if w >= 2:
                # Read a byte exp(w-2)'s LAST chunk wrote: the single wait
                # "ACT >= exp(w-2, k3)" dominates every chunk's WAW below.
                dead3 = mpool.tile([Q, 1], BF16, tag="dead3", bufs=2)
                act_abs = nc.scalar.copy(
                    dead3, em_tiles[w - 2][:, FI - 2 : FI - 1]
                )
                add_dep_helper(act_abs.ins, dead4_insts[w - 2].ins, sync=False)

            # 4 chunks per window: exp/PE start as soon as the first quarter
            # of STTs lands, and the tail after the last DMA stays short.
            # s_t arrives as four contiguous quarters [quarter, n, QW_].
            s_v4 = s_t.rearrange("p (k n i) -> p k n i", k=4, n=NPROT)
            nchunk = 4
            cw = FI // nchunk
            s_next = None
            for k in range(nchunk):
                i0 = k * cw
                # s16 = (mne * -1e4) + d, bf16 out, 8 protos per class block
                for c in range(C):
                    n0 = 8 * c
                    mne_b = (
                        mne_v[:, c, i0 : i0 + cw]
                        .unsqueeze(1)
                        .broadcast_to([Q, 8, cw])
                    )
                    i_stt = nc.vector.scalar_tensor_tensor(
                        s16_v[:, n0 : n0 + 8, i0 : i0 + cw],
                        mne_b,
                        -1.0e4,
                        s_v4[:, k, n0 : n0 + 8, :],
                        mybir.AluOpType.mult,
                        mybir.AluOpType.add,
                    )
                    if k == 0 and c == 0:
                        add_dep_helper(i_stt.ins, dve_prev.ins, sync=False)

                act_prev = act_abs
                act_abs = None
                # ACT-side observer of the last STT of this chunk: the exp
                # below then sheds its DVE wait, and the prefetch DMA can
                # issue waitlessly right here.
                obs2 = mpool.tile([Q, 1], BF16, tag="obs2", bufs=8)
                i_obs2 = nc.scalar.copy(
                    obs2, s16_v[:, 79, i0 + cw - 1 : i0 + cw]
                )
                if act_prev is not None:
                    add_dep_helper(i_obs2.ins, act_prev.ins, sync=False)
                act_prev = i_obs2
                if k == nchunk - 1 and w + 2 < W:
                    # All STTs of this window are done; prefetch window w+2
                    # (two dma_starts of two quarter-blocks each) into the
                    # freed buffer.  The 1-elem copies absorb the four old
                    # quarter-transfers' DMAHW ticks so the dma issues carry
                    # no waits (the DVE tick came via obs2).
                    s_next = spool.tile(
                        [Q, NPROT * FI], F32, tag="s", name=f"s_t{w+2}"
                    )
                    s_tiles.append(s_next)
                    prev = i_obs2
                    for k2 in range(4):
                        dmaobs = mpool.tile([Q, 1], F32, tag="dmaobs", bufs=8)
                        i_do = nc.scalar.copy(
                            dmaobs, s_t[:, k2 * QB : k2 * QB + 1]
                        )
                        add_dep_helper(i_do.ins, prev.ins, sync=False)
                        prev = i_do
                    din_v = d_in.rearrange("(a q) m -> a q m", q=Q)
                    for h2 in range(2):
                        a0 = 4 * (w + 2) + 2 * h2
                        i_dma = nc.scalar.dma_start(
                            out=s_next[
                                :, 2 * h2 * QB : 2 * (h2 + 1) * QB
                            ].rearrange("p (x m) -> p x m", x=2),
                            in_=din_v[a0 : a0 + 2].transpose([1, 0, 2]),
                        )
                        add_dep_helper(i_dma.ins, prev.ins, sync=False)
                        prev = i_dma
                    act_prev = prev

                # em = exp(s16), bf16
                i_exp = nc.scalar.activation(
                    em_v[:, :, i0 : i0 + cw],
                    s16_v[:, :NPROT, i0 : i0 + cw],
                    mybir.ActivationFunctionType.Exp,
                )
                if act_prev is not None:
                    add_dep_helper(i_exp.ins, act_prev.ins, sync=False)

                for i in range(i0, i0 + cw):
                    nc.tensor.matmul(
                        g_ps,
                        s16_v[:, :, i],
                        em_v[:, :, i],
                        start=first,
                        stop=(w == W - 1 and i == FI - 1),
                    )
                    first = False

            if w + 2 < W:
                # Read the accumulator right after this window's last matmul:
                # the copy waits exactly on "PE done with window w", putting
                # that tick into ACT's clock for window w+2's exp.
                dead4 = mpool.tile([1, 1], F32, tag="dead4", bufs=2)
                dead4_insts.append(nc.scalar.copy(dead4, g_ps[0:1, 0:1]))
                dead4_tiles.append(dead4)

        # Absorb the ACT-PSUM-read serialization into DVE so the final
        # PSUM->SBUF copy carries only the PE wait.
        deadf = mpool.tile([1, 1], F32, tag="deadf", bufs=1)
        i_deadf = nc.vector.tensor_copy(deadf, dead4_tiles[-1])
        g_sb = singles.tile([NSLOT, NPROT], F32)
        i_gcopy = nc.vector.tensor_copy(g_sb, g_ps)
        add_dep_helper(i_gcopy.ins, i_deadf.ins, sync=False)
        # Output DMA from ACT behind a g_sb observer, so the issue carries
        # at most the DMAHW semaphore-recycling wait.
        gobs = mpool.tile([1, 1], F32, tag="gobs", bufs=1)
        i_gobs = nc.scalar.copy(gobs, g_sb[0:1, 0:1])
        add_dep_helper(i_gobs.ins, i_gcopy.ins, sync=False)
        i_gdma = nc.scalar.dma_start(out=g_out[:, :], in_=g_sb)
        add_dep_helper(i_gdma.ins, i_gobs.ins, sync=False)

    # The kernel-tail drain aggregates every outstanding semaphore into one
    # instruction; the CTRL struct cannot hold that many waits.  Split it
    # into a chain of single-wait drains.
    import copy as _copy

    for fn in nc.m.functions:
        for blk in fn.blocks:
            insts = blk.instructions
            for idx, ins in enumerate(list(insts)):
                si = ins.sync_info
                if type(ins).__name__ == "InstDrain" and si and len(si.on_wait) > 1:
                    waits = list(si.on_wait)
                    si.on_wait = waits[-1:]
                    pos = insts.index(ins)
                    for k, wt in enumerate(waits[:-1]):
                        d2 = _copy.deepcopy(ins)
                        d2.name = f"{ins.name}-split{k}"
                        d2.sync_info = type(si)(on_wait=[wt], on_update=[])
                        insts.insert(pos + k, d2)
                    break

    return nc


def _get_nc():
    if "nc" not in _NC_CACHE:
        _NC_CACHE["nc"] = build_nc()
    return _NC_CACHE["nc"]


def run_device(dist8, labf8, trace=False):
    """dist8: [8, W*Q, NPROT*FI] f32 device layout; labf8: [8, P] f32 labels-1."""
    nc = _get_nc()
    cls = np.broadcast_to(np.arange(C, dtype=np.float32)[None, :], (Q, C))
    in_maps = []
    for b in range(B):
        labcls = np.concatenate([labf8[b].reshape(Q, 512), cls], axis=1)
        in_maps.append(
            {"dist": dist8[b], "labcls": np.ascontiguousarray(labcls)}
        )
    return run_bass_kernel_spmd(nc, in_maps, list(range(B)), trace=trace)


def kernel(
    prototype_distances,
    target_labels,
    proto_class,
    pair_i,
    pair_j,
    pair_cls,
    _trace=False,
    _results_out=None,
):
    dist = np.asarray(prototype_distances, dtype=np.float32).reshape(B, NPROT, P)
    labels = np.asarray(target_labels).reshape(B, P).astype(np.int64)
    proto_class = np.asarray(proto_class, dtype=np.int64)
    pair_i = np.asarray(pair_i, dtype=np.int64)
    pair_j = np.asarray(pair_j, dtype=np.int64)
    pair_cls = np.asarray(pair_cls, dtype=np.int64)

    # Permute prototypes to class-major layout: slot n -> class n // 8.
    perm = np.empty(NPROT, dtype=np.int64)
    for c in range(C):
        protos = np.nonzero(proto_class == c)[0]
        assert len(protos) == 8, "expect 8 prototypes per class"
        perm[8 * c : 8 * c + 8] = protos
    inv = np.empty(NPROT, dtype=np.int64)
    inv[perm] = np.arange(NPROT)

    # Device layout [w, quarter, q, n, i]: pixel p = 512*q + 128*w + 32*k + i,
    # protos class-major.  One transpose+copy host-side buys fully
    # contiguous quarter-window device DMAs.
    QW_ = FI // 4
    dist_v = dist[:, perm, :].reshape(B, NPROT, Q, W, 4, QW_)
    dist_p = np.ascontiguousarray(dist_v.transpose(0, 3, 4, 2, 1, 5)).reshape(
        B, W * 4 * Q, NPROT * QW_
    )
    labf = np.ascontiguousarray((labels - 1).astype(np.float32))

    br = run_device(dist_p, labf, trace=_trace)
    if _results_out is not None:
        _results_out.append(br)

    total_vals = np.float64(0.0)
    total_valid = 0
    for b in range(B):
        out = br.results[b]["g"]  # [81, 80]; out[j, a] = G[a, j], out[80, a] = Z_a
        Z = out[NPROT].astype(np.float64)
        Gt = out[:NPROT].astype(np.float64)  # Gt[j, a] = sum_p em_a * s_j
        with np.errstate(divide="ignore", invalid="ignore"):
            A = np.where(Z[None, :] != 0.0, Gt / Z[None, :], 0.0)  # A[j, a] = E_a[d_j]
        lb = labels[b] - 1
        cnt = np.bincount(lb[lb >= 0], minlength=C)
        ii = inv[pair_i]
        jj = inv[pair_j]
        # A[x, a] = expectation of d_x under softmax of proto a
        kld = 0.5 * (A[jj, jj] - A[jj, ii] + A[ii, ii] - A[ii, jj])
        valid = cnt[pair_cls] >= 2
        total_vals += np.exp(-kld[valid]).sum()
        total_valid += int(valid.sum())

    if total_valid > 0:
        res = np.float32(total_vals / max(total_valid, 1))
    else:
        res = np.float32(0.0)
    return res


if __name__ == "__main__":
    rng = np.random.default_rng(0)
    d = rng.standard_normal((B, NPROT, 256, 256), dtype=np.float32)
    l = rng.integers(0, 11, (B, 256, 256))
    pc = (np.arange(NPROT) % 40) // 4
    pairs = []
    for s in range(2):
        for c in range(C):
            base = s * 40 + c * 4
            for a in range(4):
                for b2 in range(a + 1, 4):
                    pairs.append((base + a, base + b2, c))
    pairs = np.asarray(pairs, np.int32)
    print(kernel(d, l, pc, pairs[:, 0], pairs[:, 1], pairs[:, 2]))


# revision 38
# speedup vs baseline: 2.8394x; 1.2908x over previous
"""Trainium2 Bass kernel for nn_KLDLoss_18769007083961.

Math reformulation (validated vs reference, rel err ~3e-5 with bf16):
  For each image b, prototype a with class c(a), define over pixels p:
    s_a[p]  = d_a[p] + (label[p] != c(a)) * (-1e4)      # masked-biased distance
    em_a[p] = exp(s_a[p])                               # exactly 0 off-class (underflow)
    Z_a     = sum_p em_a[p]
    G[a,j]  = sum_p em_a[p] * s_j[p]   (j in same group => same class mask)
    A[a,j]  = G[a,j] / Z_a
  Symmetric KL for a same-group pair (i,j) (log-partition terms cancel):
    kld = 0.5 * (A[j,j] - A[j,i] + A[i,i] - A[i,j])
  loss = mean over valid pairs (class count >= 2) of exp(-kld).

Only same-class G entries are consumed, and em is exactly zero off-class,
so the bf16 rounding of the -1e4 bias in s never reaches the result: the
biased s tile can be written once in bf16 and feed both the exp (ACT) and
the matmul lhsT (PE).  bf16 matmul runs at 1 cycle/row vs fp32's 4.

Device kernel (one image per NeuronCore, 8 cores):
  Layout: pixel p = 512*q + 128*w + i  (q = SBUF partition, w = window, i = inner).
  Protos are permuted host-side to class-major order (slot = 8c + 4s + m) so the
  class bias applies to 8 contiguous protos per DVE op.
  Per window: DMA dist -> s_t[128, 80*128] f32; DVE builds (label != c) and
  writes s16[128, 81*128] bf16 = d + mne*-1e4 (slot 80 memset to 1.0);
  ACT computes em = exp(s16) bf16; 128 matmuls (lhsT = s16-slice [128,81],
  rhs = em-slice [128,80]) accumulate out[j,a] = G[a,j], row 80 = Z, in
  PSUM [81,80] f32.  The last window runs in 4 column chunks to shorten the
  pipeline tail after the final DMA.  Host does the tiny 120-pair combine.
"""

import sys
from contextlib import ExitStack

import numpy as np

sys.path.insert(0, "/opt/trn_rl_repo")

import concourse.bass as bass
import concourse.tile as tile
from concourse import mybir
from concourse.bass_utils import run_bass_kernel_spmd
from concourse.tile import add_dep_helper

B = 8
C = 10
NPROT = 80
NSLOT = NPROT + 1  # 80 protos + ones column for Z
P = 65536
Q = 128          # partitions = coarse pixel blocks of 512
W = 4            # windows per image
FI = 128         # inner pixels per window per partition
F32 = mybir.dt.float32
BF16 = mybir.dt.bfloat16

_NC_CACHE = {}


def build_nc():
    nc = bass.Bass()
    # dist pre-transposed host-side to [w, quarter, q, n, i]: every quarter-
    # window DMA is a plain contiguous copy, so the first STT chunk starts
    # after ~1/16 of the image has landed.
    d_in = nc.dram_tensor(
        "dist", [W * 4 * Q, NPROT * (FI // 4)], F32, kind="ExternalInput"
    )
    # labels [q, 512] packed with the 10 class constants -> cols 512..521
    lab_in = nc.dram_tensor("labcls", [Q, 512 + C], F32, kind="ExternalInput")
    g_out = nc.dram_tensor("g", [NSLOT, NPROT], F32, kind="ExternalOutput")

    with ExitStack() as ctx:
        tc = ctx.enter_context(tile.TileContext(nc))
        singles = ctx.enter_context(tc.tile_pool(name="singles", bufs=1))
        spool = ctx.enter_context(tc.tile_pool(name="spool", bufs=2))
        s16pool = ctx.enter_context(tc.tile_pool(name="s16pool", bufs=2))
        empool = ctx.enter_context(tc.tile_pool(name="empool", bufs=2))
        mpool = ctx.enter_context(tc.tile_pool(name="mpool", bufs=2))
        psum = ctx.enter_context(tc.tile_pool(name="psum", bufs=1, space="PSUM"))

        labels_t = singles.tile([Q, 512 + C], F32)
        nc.sync.dma_start(out=labels_t, in_=lab_in[:, :])
        cls_t = labels_t[:, 512 : 512 + C]

        g_ps = psum.tile([NSLOT, NPROT], F32)

        QW_ = FI // 4  # pixels per quarter-window
        QB = NPROT * QW_  # sbuf columns per quarter block
        s_tiles = []
        for w in range(2):
            s_w = spool.tile([Q, NPROT * FI], F32, tag="s", name=f"s_t{w}")
            for k in range(4):
                nc.sync.dma_start(
                    out=s_w[:, k * QB : (k + 1) * QB],
                    in_=d_in[(4 * w + k) * Q : (4 * w + k + 1) * Q, :],
                )
            s_tiles.append(s_w)

        # constant source for the ACT-side absorber writes below
        zconst = singles.tile([Q, 1], BF16)
        nc.vector.memset(zconst, 0.0)

        # Engines have a single sync-wait slot per instruction.  Every
        # cross-engine dependency is therefore carried by a dedicated
        # 1-element absorber op, pinned ahead of its consumer with no-sync
        # dep edges so the scheduler keeps the elision-enabling order.
        first = True
        em_tiles = []
        dead4_tiles = []
        dead4_insts = []
        for w in range(W):
            s_t = s_tiles[w]

            # mne[p, c, i] = (labels != c) as 1.0/0.0, bf16
            mne = mpool.tile([Q, C * FI], BF16, tag="mne")
            mne_v = mne.rearrange("p (c i) -> p c i", c=C)
            lab_w = labels_t[:, w * FI : (w + 1) * FI]
            nc.vector.tensor_tensor(
                mne_v,
                lab_w.unsqueeze(1).broadcast_to([Q, C, FI]),
                cls_t.unsqueeze(2).broadcast_to([Q, C, FI]),
                mybir.AluOpType.not_equal,
            )

            # DVE absorber chain: (1) dist-DMA completion for this window
            probe = mpool.tile([Q, 1], F32, tag="probe", bufs=4)
            i_probe = nc.vector.tensor_copy(probe, s_t[:, 0:1])
            dve_prev = i_probe
            if w >= 2:
                # (2) ACT finished exp(w-2) (read byte from its LAST chunk),
                # which read the s16 buffer the STTs below recycle
                probe2 = mpool.tile([Q, 1], BF16, tag="probe2", bufs=4)
                i_probe2 = nc.vector.tensor_copy(
                    probe2,
                    em_tiles[w - 2][:, (FI - 1) * NPROT : (FI - 1) * NPROT + 1],
                )
                add_dep_helper(i_probe2.ins, dve_prev.ins, sync=False)
                dve_prev = i_probe2

            s16 = s16pool.tile([Q, NSLOT * FI], BF16, tag="s16")
            s16_v = s16.rearrange("p (i n) -> p i n", n=NSLOT)
            em = empool.tile([Q, NPROT * FI], BF16, tag="em")
            em_v = em.rearrange("p (i n) -> p i n", n=NPROT)
            em_tiles.append(em)

            # ones column (slot 80) -> Z row of the gram.  For w >= 2 its
            # bytes were read by every LDW of window w-2, so this memset
            # carries exactly the "PE done with window w-2" wait that the
            # STTs would otherwise each need.
            i_memset = nc.vector.memset(s16_v[:, :, NPROT], 1.0)
            add_dep_helper(i_memset.ins, dve_prev.ins, sync=False)
            dve_prev = i_memset

            # ACT absorber: reading an old-em byte absorbs the same-engine
            # WAW tick (exp(w) overwrites exp(w-2)'s output); the PE tick was
            # absorbed by dead_act at the end of window w-2.
            act_abs = None
            if w >= 2:
                # Read a byte exp(w-2)'s LAST chunk wrote: the single wait
                # "ACT >= exp(w-2, k3)" dominates every chunk's WAW below.
                dead3 = mpool.tile([Q, 1], BF16, tag="dead3", bufs=2)
                act_abs = nc.scalar.copy(
                    dead3,
                    em_tiles[w - 2][
                        :, (FI - 1) * NPROT + 1 : (FI - 1) * NPROT + 2
                    ],
                )
                add_dep_helper(act_abs.ins, dead4_insts[w - 2].ins, sync=False)

            # 4 chunks per window: exp/PE start as soon as the first quarter
            # of STTs lands, and the tail after the last DMA stays short.
            # s_t arrives as four contiguous quarters [quarter, n, QW_].
            s_v4 = s_t.rearrange("p (k i n) -> p k i n", k=4, i=FI // 4)
            nchunk = 4
            cw = FI // nchunk
            s_next = None
            for k in range(nchunk):
                i0 = k * cw
                # s16 = (mne * -1e4) + d, bf16 out, 8 protos per class block
                for c in range(C):
                    n0 = 8 * c
                    mne_b = (
                        mne_v[:, c, i0 : i0 + cw]
                        .unsqueeze(2)
                        .broadcast_to([Q, cw, 8])
                    )
                    i_stt = nc.vector.scalar_tensor_tensor(
                        s16_v[:, i0 : i0 + cw, n0 : n0 + 8],
                        mne_b,
                        -1.0e4,
                        s_v4[:, k, :, n0 : n0 + 8],
                        mybir.AluOpType.mult,
                        mybir.AluOpType.add,
                    )
                    if k == 0 and c == 0:
                        add_dep_helper(i_stt.ins, dve_prev.ins, sync=False)

                act_prev = act_abs
                act_abs = None
                # ACT-side observer of the last STT of this chunk: the exp
                # below then sheds its DVE wait, and the prefetch DMA can
                # issue waitlessly right here.
                obs2 = mpool.tile([Q, 1], BF16, tag="obs2", bufs=8)
                i_obs2 = nc.scalar.copy(
                    obs2,
                    s16[:, (i0 + cw - 1) * NSLOT + 79 : (i0 + cw - 1) * NSLOT + 80],
                )
                if act_prev is not None:
                    add_dep_helper(i_obs2.ins, act_prev.ins, sync=False)
                act_prev = i_obs2
                if k == nchunk - 1 and w + 2 < W:
                    # All STTs of this window are done; prefetch window w+2
                    # (two dma_starts of two quarter-blocks each) into the
                    # freed buffer.  The 1-elem copies absorb the four old
                    # quarter-transfers' DMAHW ticks so the dma issues carry
                    # no waits (the DVE tick came via obs2).
                    s_next = spool.tile(
                        [Q, NPROT * FI], F32, tag="s", name=f"s_t{w+2}"
                    )
                    s_tiles.append(s_next)
                    prev = i_obs2
                    for k2 in range(4):
                        dmaobs = mpool.tile([Q, 1], F32, tag="dmaobs", bufs=8)
                        i_do = nc.scalar.copy(
                            dmaobs, s_t[:, k2 * QB : k2 * QB + 1]
                        )
                        add_dep_helper(i_do.ins, prev.ins, sync=False)
                        prev = i_do
                    din_v = d_in.rearrange("(a q) m -> a q m", q=Q)
                    for h2 in range(2):
                        a0 = 4 * (w + 2) + 2 * h2
                        i_dma = nc.scalar.dma_start(
                            out=s_next[
                                :, 2 * h2 * QB : 2 * (h2 + 1) * QB
                            ].rearrange("p (x m) -> p x m", x=2),
                            in_=din_v[a0 : a0 + 2].transpose([1, 0, 2]),
                        )
                        add_dep_helper(i_dma.ins, prev.ins, sync=False)
                        prev = i_dma
                    act_prev = prev

                # em = exp(s16), bf16
                i_exp = nc.scalar.activation(
                    em_v[:, i0 : i0 + cw, :],
                    s16_v[:, i0 : i0 + cw, :NPROT],
                    mybir.ActivationFunctionType.Exp,
                )
                if act_prev is not None:
                    add_dep_helper(i_exp.ins, act_prev.ins, sync=False)

                for i in range(i0, i0 + cw):
                    nc.tensor.matmul(
                        g_ps,
                        s16_v[:, i, :],
                        em_v[:, i, :],
                        start=first,
                        stop=(w == W - 1 and i == FI - 1),
                    )
                    first = False

            if w + 2 < W:
                # Read the accumulator right after this window's last matmul:
                # the copy waits exactly on "PE done with window w", putting
                # that tick into ACT's clock for window w+2's exp.
                dead4 = mpool.tile([1, 1], F32, tag="dead4", bufs=2)
                dead4_insts.append(nc.scalar.copy(dead4, g_ps[0:1, 0:1]))
                dead4_tiles.append(dead4)

        # Absorb the ACT-PSUM-read serialization into DVE so the final
        # PSUM->SBUF copy carries only the PE wait.
        deadf = mpool.tile([1, 1], F32, tag="deadf", bufs=1)
        i_deadf = nc.vector.tensor_copy(deadf, dead4_tiles[-1])
        g_sb = singles.tile([NSLOT, NPROT], F32)
        i_gcopy = nc.vector.tensor_copy(g_sb, g_ps)
        add_dep_helper(i_gcopy.ins, i_deadf.ins, sync=False)
        # Output DMA from ACT behind a g_sb observer, so the issue carries
        # at most the DMAHW semaphore-recycling wait.
        gobs = mpool.tile([1, 1], F32, tag="gobs", bufs=1)
        i_gobs = nc.scalar.copy(gobs, g_sb[0:1, 0:1])
        add_dep_helper(i_gobs.ins, i_gcopy.ins, sync=False)
        i_gdma = nc.scalar.dma_start(out=g_out[:, :], in_=g_sb)
        add_dep_helper(i_gdma.ins, i_gobs.ins, sync=False)

    # The kernel-tail drain aggregates every outstanding semaphore into one
    # instruction; the CTRL struct cannot hold that many waits.  Split it
    # into a chain of single-wait drains.
    import copy as _copy

    for fn in nc.m.functions:
        for blk in fn.blocks:
            insts = blk.instructions
            for idx, ins in enumerate(list(insts)):
                si = ins.sync_info
                if type(ins).__name__ == "InstDrain" and si and len(si.on_wait) > 1:
                    waits = list(si.on_wait)
                    si.on_wait = waits[-1:]
                    pos = insts.index(ins)
                    for k, wt in enumerate(waits[:-1]):
                        d2 = _copy.deepcopy(ins)
                        d2.name = f"{ins.name}-split{k}"
                        d2.sync_info = type(si)(on_wait=[wt], on_update=[])
                        insts.insert(pos + k, d2)
                    break

    return nc


def _get_nc():
    if "nc" not in _NC_CACHE:
        _NC_CACHE["nc"] = build_nc()
    return _NC_CACHE["nc"]


def run_device(dist8, labf8, trace=False):
    """dist8: [8, W*Q, NPROT*FI] f32 device layout; labf8: [8, P] f32 labels-1."""
    nc = _get_nc()
    cls = np.broadcast_to(np.arange(C, dtype=np.float32)[None, :], (Q, C))
    in_maps = []
    for b in range(B):
        labcls = np.concatenate([labf8[b].reshape(Q, 512), cls], axis=1)
        in_maps.append(
            {"dist": dist8[b], "labcls": np.ascontiguousarray(labcls)}
        )
    return run_bass_kernel_spmd(nc, in_maps, list(range(B)), trace=trace)


def kernel(
    prototype_distances,
    target_labels,
    proto_class,
    pair_i,
    pair_j,
    pair_cls,
    _trace=False,
    _results_out=None,
):
    dist = np.asarray(prototype_distances, dtype=np.float32).reshape(B, NPROT, P)
    labels = np.asarray(target_labels).reshape(B, P).astype(np.int64)
    proto_class = np.asarray(proto_class, dtype=np.int64)
    pair_i = np.asarray(pair_i, dtype=np.int64)
    pair_j = np.asarray(pair_j, dtype=np.int64)
    pair_cls = np.asarray(pair_cls, dtype=np.int64)

    # Permute prototypes to class-major layout: slot n -> class n // 8.
    perm = np.empty(NPROT, dtype=np.int64)
    for c in range(C):
        protos = np.nonzero(proto_class == c)[0]
        assert len(protos) == 8, "expect 8 prototypes per class"
        perm[8 * c : 8 * c + 8] = protos
    inv = np.empty(NPROT, dtype=np.int64)
    inv[perm] = np.arange(NPROT)

    # Device layout [w, quarter, q, n, i]: pixel p = 512*q + 128*w + 32*k + i,
    # protos class-major.  One transpose+copy host-side buys fully
    # contiguous quarter-window device DMAs.
    QW_ = FI // 4
    dist_v = dist[:, perm, :].reshape(B, NPROT, Q, W, 4, QW_)
    dist_p = np.ascontiguousarray(dist_v.transpose(0, 3, 4, 2, 5, 1)).reshape(
        B, W * 4 * Q, NPROT * QW_
    )
    labf = np.ascontiguousarray((labels - 1).astype(np.float32))

    br = run_device(dist_p, labf, trace=_trace)
    if _results_out is not None:
        _results_out.append(br)

    total_vals = np.float64(0.0)
    total_valid = 0
    for b in range(B):
        out = br.results[b]["g"]  # [81, 80]; out[j, a] = G[a, j], out[80, a] = Z_a
        Z = out[NPROT].astype(np.float64)
        Gt = out[:NPROT].astype(np.float64)  # Gt[j, a] = sum_p em_a * s_j
        with np.errstate(divide="ignore", invalid="ignore"):
            A = np.where(Z[None, :] != 0.0, Gt / Z[None, :], 0.0)  # A[j, a] = E_a[d_j]
        lb = labels[b] - 1
        cnt = np.bincount(lb[lb >= 0], minlength=C)
        ii = inv[pair_i]
        jj = inv[pair_j]
        # A[x, a] = expectation of d_x under softmax of proto a
        kld = 0.5 * (A[jj, jj] - A[jj, ii] + A[ii, ii] - A[ii, jj])
        valid = cnt[pair_cls] >= 2
        total_vals += np.exp(-kld[valid]).sum()
        total_valid += int(valid.sum())

    if total_valid > 0:
        res = np.float32(total_vals / max(total_valid, 1))
    else:
        res = np.float32(0.0)
    return res


if __name__ == "__main__":
    rng = np.random.default_rng(0)
    d = rng.standard_normal((B, NPROT, 256, 256), dtype=np.float32)
    l = rng.integers(0, 11, (B, 256, 256))
    pc = (np.arange(NPROT) % 40) // 4
    pairs = []
    for s in range(2):
        for c in range(C):
            base = s * 40 + c * 4
            for a in range(4):
                for b2 in range(a + 1, 4):
                    pairs.append((base + a, base + b2, c))
    pairs = np.asarray(pairs, np.int32)
    print(kernel(d, l, pc, pairs[:, 0], pairs[:, 1], pairs[:, 2]))
